# revision 26
# baseline (speedup 1.0000x reference)
"""Trainium2 Bass kernel for policy-weighted multi-head attention.

Reference computation (per batch b, 8 batches):
    qkv = x @ qkv_w.T                     # [N, 3*H*HD]
    q, k, v per head                      # H=12 heads, HD=64
    s = (q * HD^-0.5) @ k.T               # [N, N]
    a[n,m] ~ exp(s[n,m]) * (pol[m] + (1-pol[m])*eye)  normalized over m
    out = a @ v ; y = out @ proj_w.T + b

Sharding: pure data parallel, one batch per NeuronCore (8 cores).

Kernel strategy (per core):
  - Host pre-transposes x, qkv_w, proj_w so no on-chip transposes are needed.
  - All matmuls run as float32r (full fp32 data, ~250ns per [128x128x512]).
  - Attention runs in the S^T layout (partitions = key index m): the softmax
    sum over m folds into the PE via an appended ones column on the
    (policy-prescaled) V; the denominator appears as row 64 of the
    attention-output matmul.
  - The policy multiply is folded into V (rows pre-scaled by pol[m]); the
    diagonal term becomes masked multiplies with a precomputed
    [128, 8, 128] mask whose diagonal is 1/pol.
  - nh-major scheduling: the two 512-wide halves of the query axis are
    processed as outer passes over all 12 heads; all 6 q/k pair buffers stay
    resident (prefetched during pass 0), and the projection matmuls for
    pass-0 rows are interleaved into pass 1 so the PE never drains between
    attention and projection.
  - Normalization is DMA-free: per (head, half), 1/denominator via a single
    fast-approx DVE reciprocal on the av row, partition-broadcast by a K=1
    ones-matmul into PSUM, then one DVE multiply into the output tile.  Only
    the odd-head partition shift (rows 64..127) needs an SBUF->SBUF DMA, on
    the otherwise-idle SWDGE ring.  Norm emission is deferred into the next
    head's st loop so the PE never waits on the DVE reciprocal chain.
  - Bulk inputs are single-dispatch DMAs (each dma_start costs ~0.65us of
    sequencer dispatch); the first qk pair + x streams go on the Act HWDGE
    ring, which is idle during the sync ring's kernel preamble, and dummy
    warm-up matmuls run during the load so the PE clock-gate (HAM) reaches
    2.4GHz before real work starts.
  - max-subtraction and the eps terms of the reference softmax are dropped:
    logits are ~N(0,1) so exp() cannot overflow, and the eps corrections
    are ~1e-9 relative -- far below fp32 noise.
"""

import os

os.environ.setdefault("JAX_PLATFORMS", "axon")

from contextlib import ExitStack

import ml_dtypes
import numpy as np

import concourse.bass as bass
import concourse.tile as tile
from concourse import bacc, mybir
from concourse.bass_utils import run_bass_kernel_spmd

B, N, C = 8, 1024, 768
H, HD = 12, 64
SCALE = HD ** (-0.5)
F32 = mybir.dt.float32
F32R = mybir.dt.float32r
BF16 = mybir.dt.bfloat16
P = 128
NC_ = N // P  # 8 seq chunks
CC = C // P  # 6 channel chunks
NH = N // 512  # 2 free-dim halves of the seq axis

LAST_RESULTS = None  # BassKernelResults of the most recent run (for test.py)


def _build_nc():
    nc = bacc.Bacc(None, target_bir_lowering=False)

    xT_d = nc.dram_tensor("xT", [C, N], F32R, kind="ExternalInput")
    wqkT_d = nc.dram_tensor("wqkT", [C, 2 * H * HD], F32R, kind="ExternalInput")
    wvT_d = nc.dram_tensor("wvT", [C, H * HD], F32R, kind="ExternalInput")
    pwT_d = nc.dram_tensor("pwT", [C, C], F32R, kind="ExternalInput")
    bias_d = nc.dram_tensor("bias", [C], F32, kind="ExternalInput")
    polT_d = nc.dram_tensor("polT", [P, NC_], F32, kind="ExternalInput")
    dmask_d = nc.dram_tensor("dmask", [P, NC_, P], F32R, kind="ExternalInput")
    y_d = nc.dram_tensor("y", [N, C], F32, kind="ExternalOutput")

    def dram_cc(t_d, c0, w):
        # [768, w] DRAM slice viewed as [128, CC, w] for one-dispatch loads
        return t_d[:, c0 : c0 + w].rearrange("(cc p) w -> p cc w", p=P)

    with ExitStack() as ctx:
        tc = ctx.enter_context(tile.TileContext(nc))

        persist = ctx.enter_context(tc.tile_pool(name="persist", bufs=1))
        xT_sb = persist.tile([P, CC, N], F32R)
        # v in natural layout, pol-scaled, with a pol column at d=64
        v_aug = persist.tile([P, NC_, H, HD + 1], F32R)
        pw_sb = persist.tile([P, CC, C], F32R)
        b_sb = persist.tile([P, C], F32)
        polT_sb = persist.tile([P, NC_], F32)
        dmask_sb = persist.tile([P, NC_, P], F32R)
        e_f32 = persist.tile([P, HD], F32)
        e_sb = persist.tile([P, HD], F32R)  # unit row 64: selects 1/den
        z_f32 = persist.tile([P, 512], F32)
        rcp_rs = [persist.tile([P, 512], F32R, name=f"rcp_r{i}") for i in (0, 1)]
        outT = persist.tile([P, CC, N], F32R)

        qkp = ctx.enter_context(tc.tile_pool(name="qkp", bufs=1))
        ps_st = ctx.enter_context(tc.tile_pool(name="ps_st", bufs=3, space="PSUM"))
        ps_av = ctx.enter_context(tc.tile_pool(name="ps_av", bufs=1, space="PSUM"))
        ps_mx = ctx.enter_context(tc.tile_pool(name="ps_mx", bufs=1, space="PSUM"))

        nc.vector.memset(e_f32, 0.0)
        nc.vector.memset(e_f32[HD : HD + 1, :], 1.0)
        nc.vector.tensor_copy(out=e_sb, in_=e_f32)
        # rows 65..127 of the broadcast rhs must be real zeros: the K=128
        # matmul streams all 128 partitions on hardware
        nc.vector.memset(z_f32, 0.0)
        for t in rcp_rs:
            nc.vector.tensor_copy(out=t, in_=z_f32)

        # --- priority startup on the Act ring: its sequencer is idle while
        # the sync ring runs the kernel preamble (~7us), and each dma_start
        # costs ~0.65us of sequencer dispatch, so these are one-per-tensor.
        wqk_ts = {0: qkp.tile([P, CC, 2, P], F32R, tag="wqk", bufs=3, name="wqk_t")}
        qk_ts = {}
        for kk, j in ((0, 0), (1, CC)):
            nc.scalar.dma_start(
                out=wqk_ts[0][:, :, kk, :], in_=dram_cc(wqkT_d, j * P, P)
            )
        for xh in (0, 1):
            nc.scalar.dma_start(
                out=xT_sb[:, 3 * xh : 3 * xh + 3, 0:512],
                in_=xT_d[xh * 3 * P : (xh + 1) * 3 * P, 0:512].rearrange(
                    "(cc p) w -> p cc w", p=P
                ),
            )
        # bulk on the sync ring, in need order
        nc.sync.dma_start(out=polT_sb, in_=polT_d[:])

        # HAM warm-up: the PE clock-gate defaults to 1.2GHz and only reaches
        # 2.4GHz after ~3.4us of sustained matmul activity.  Burn dummy
        # matmuls on the first-arriving weight tile while the startup burst
        # streams in, so real matmuls run at full clock.
        with nc.named_scope("warmup"):
            ps_w = ps_mx.tile([P, 512], F32, tag="mx", name="ps_warm")
            for _ in range(26):
                nc.tensor.matmul(
                    ps_w[:, 0:128],
                    lhsT=wqk_ts[0][:, 0, 0, :],
                    rhs=wqk_ts[0][:, 0, 0, :],
                    start=True,
                    stop=True,
                )

        def emit_qk_mms(jq, nhs=(0, 1)):
            """qk^T matmuls for pair jq: qk_t[:,0,:] = q chunk jq,
            qk_t[:,1,:] = k chunk jq+6 (embedding dim on partitions)."""
            wqk_t, qk_t = wqk_ts[jq], qk_ts[jq]
            with nc.named_scope("qk_mm"):
                for kk in range(2):
                    for nh in nhs:
                        ps = ps_mx.tile([P, 512], F32, tag="mx", name="ps_qk")
                        for cc in range(CC):
                            nc.tensor.matmul(
                                ps,
                                lhsT=wqk_t[:, cc, kk, :],
                                rhs=xT_sb[:, cc, nh * 512 : (nh + 1) * 512],
                                start=(cc == 0),
                                stop=(cc == CC - 1),
                            )
                        nc.scalar.activation(
                            out=qk_t[:, kk, nh * 512 : (nh + 1) * 512],
                            in_=ps,
                            func=mybir.ActivationFunctionType.Copy,
                        )

        qk_ts[0] = qkp.tile([P, 2, N], BF16, tag="qkT", bufs=CC, name="qk_t")
        emit_qk_mms(0, nhs=(0,))
        # pol columns of v_aug: DVE free-dim broadcast copies
        for nch in range(NC_):
            nc.vector.tensor_copy(
                out=v_aug[:, nch, :, HD : HD + 1],
                in_=polT_sb[:, nch : nch + 1]
                .unsqueeze(1)
                .broadcast_to((P, H, 1)),
            )
        # ---- v natural layout, pol-scaled, into v_aug --------------------
        with tc.tile_pool(name="phv", bufs=1) as phv:
            wv_sb = phv.tile([P, CC, H * HD], F32R)
            nc.sync.dma_start(out=wv_sb, in_=dram_cc(wvT_d, 0, H * HD))
            nc.sync.dma_start(
                out=xT_sb[:, :, 512:1024], in_=dram_cc(xT_d, 512, 512)
            )
            nc.sync.dma_start(out=dmask_sb, in_=dmask_d[:])
            nc.sync.dma_start(out=b_sb, in_=bias_d[:].partition_broadcast(P))

            def v_mm(nchs):
                with nc.named_scope("v_mm"):
                    for nch in nchs:
                        for ev0, ev_sz, h0 in ((0, 512, 0), (512, 256, 8)):
                            nheads = ev_sz // HD
                            ps = ps_st.tile(
                                [P, 2, 512], F32, tag="st", name="ps_v"
                            )
                            psv = ps[:, 0, :ev_sz]
                            for cc in range(CC):
                                nc.tensor.matmul(
                                    psv,
                                    lhsT=xT_sb[:, cc, nch * P : (nch + 1) * P],
                                    rhs=wv_sb[:, cc, ev0 : ev0 + ev_sz],
                                    start=(cc == 0),
                                    stop=(cc == CC - 1),
                                )
                            nc.vector.tensor_mul(
                                out=v_aug[:, nch, h0 : h0 + nheads, 0:HD],
                                in0=psv.rearrange("p (h d) -> p h d", d=HD),
                                in1=polT_sb[:, nch : nch + 1]
                                .unsqueeze(1)
                                .broadcast_to((P, nheads, HD)),
                            )

            # v chunks 0-3 need only x half 0; qk pair0's nh1 matmuls slot
            # in once x half 1 lands
            v_mm(range(0, 4))
            emit_qk_mms(0, nhs=(1,))
            v_mm(range(4, NC_))

        # ------------------- attention, nh-major --------------------------
        with tc.tile_pool(name="attn", bufs=1) as attn:
            pending_norm = [None]  # deferred norm closure for the prev head

            def proj_chunk(nch):
                with nc.named_scope("proj_mm"):
                    y_t = attn.tile([P, C], F32, tag="y", bufs=2, name="y_t")
                    for oi, (o0, o_sz) in enumerate(((0, 512), (512, 256))):
                        pool, tagn = (ps_mx, "mx") if oi == 0 else (ps_av, "av")
                        ps = pool.tile([P, 512], F32, tag=tagn, name="ps_yt")
                        psy = ps[:, :o_sz]
                        for ec in range(CC):
                            nc.tensor.matmul(
                                psy,
                                lhsT=outT[:, ec, nch * P : (nch + 1) * P],
                                rhs=pw_sb[:, ec, o0 : o0 + o_sz],
                                start=(ec == 0),
                                stop=(ec == CC - 1),
                            )
                        nc.vector.tensor_add(
                            out=y_t[:, o0 : o0 + o_sz],
                            in0=psy,
                            in1=b_sb[:, o0 : o0 + o_sz],
                        )
                    nc.sync.dma_start(out=y_d[nch * P : (nch + 1) * P, :], in_=y_t)

            for nh in range(NH):
                nsl = slice(nh * 512, (nh + 1) * 512)
                for h in range(H):
                    j = h // 2
                    hp = 64 * (h % 2)
                    if nh == 0 and h % 2 == 0 and j + 1 < CC:
                        # prefetch next q/k pair (DMA + matmuls) during pass 0
                        jn = j + 1
                        wqk_ts[jn] = qkp.tile(
                            [P, CC, 2, P], F32R, tag="wqk", bufs=3, name="wqk_t"
                        )
                        for kk, jj in ((0, jn), (1, jn + CC)):
                            nc.sync.dma_start(
                                out=wqk_ts[jn][:, :, kk, :],
                                in_=dram_cc(wqkT_d, jj * P, P),
                            )
                        qk_ts[jn] = qkp.tile(
                            [P, 2, N], BF16, tag="qkT", bufs=CC, name="qk_t"
                        )
                        emit_qk_mms(jn)
                    qk_pair = qk_ts[j]
                    av = ps_av.tile([HD + 1, 512], F32, tag="av", name="av")
                    E_ts = []
                    # chunk-pair pipeline: st(t) || exp(t-1..2) || av(t-3)
                    for t in range(NC_ // 2 + 3):
                        if t == 3 and pending_norm[0] is not None:
                            # prev head's norm, emitted once this head's st
                            # matmuls are queued so the PE never waits on it
                            pending_norm[0]()
                            pending_norm[0] = None
                        if t < NC_ // 2:
                            st = ps_st.tile([P, 2, 512], F32, tag="st", name="st")
                            E_t = attn.tile(
                                [P, 2, 512], F32R, tag="E", bufs=5, name="E_t"
                            )
                            with nc.named_scope("st_mm"):
                                for k in range(2):
                                    mc = 2 * t + k
                                    nc.tensor.matmul(
                                        st[:, k, :],
                                        lhsT=qk_pair[
                                            hp : hp + HD, 1, mc * P : (mc + 1) * P
                                        ],
                                        rhs=qk_pair[hp : hp + HD, 0, nsl],
                                        start=True,
                                        stop=True,
                                    )
                            nc.scalar.activation(
                                out=E_t,
                                in_=st,
                                func=mybir.ActivationFunctionType.Exp,
                                scale=SCALE,
                            )
                            if t in (2 * nh, 2 * nh + 1):
                                # chunks 2t, 2t+1 hold the diagonal
                                diag_ap = bass.AP(
                                    tensor=E_t.tensor,
                                    offset=E_t.offset + (2 * t * P - 512 * nh),
                                    ap=[E_t.ap[0], [512 + P, 2], [1, P]],
                                )
                                nc.vector.tensor_mul(
                                    out=diag_ap,
                                    in0=diag_ap,
                                    in1=dmask_sb[:, 2 * t : 2 * t + 2, :],
                                )
                            E_ts.append(E_t)
                        if t >= 3:
                            ta = t - 3
                            E_a = E_ts[ta]
                            with nc.named_scope("av_mm"):
                                for k in range(2):
                                    mc = 2 * ta + k
                                    nc.tensor.matmul(
                                        av,
                                        lhsT=v_aug[:, mc, h, :],
                                        rhs=E_a[:, k, :],
                                        start=(mc == 0),
                                        stop=(mc == NC_ - 1),
                                    )
                    av_sb = attn.tile(
                        [HD + 1, 512], F32, tag="avsb", bufs=3, name="av_sb"
                    )
                    nc.vector.tensor_copy(out=av_sb, in_=av)

                    def make_norm(av_sb=av_sb, h=h, nh=nh, nsl=nsl):
                        def norm():
                            with nc.named_scope("norm"):
                                # rcp rows 0..63 are finite filler (the av
                                # values) so the K=65 broadcast matmul below
                                # never multiplies 0 by uninitialized bits
                                rcp = attn.tile(
                                    [HD + 1, 512], F32, tag="rcp", bufs=2,
                                    name="rcp",
                                )
                                # full-tile op from partition 0: the custom
                                # DVE op returns zeros on HW when started at
                                # a partition offset.  Rows 0..63 are 1/av --
                                # finite filler, zeroed by the selector.
                                nc.vector.reciprocal_approx_fast(
                                    out=rcp, in_=av_sb
                                )
                                # round to f32r on the (lightly loaded) Act
                                # engine; the f32r matmul requires it
                                rcp_r = rcp_rs[h % 2]
                                nc.vector.tensor_copy(
                                    out=rcp_r[0 : HD + 1, :], in_=rcp
                                )
                                r_ps = ps_mx.tile(
                                    [P, 512], F32, tag="mx", name="r_ps"
                                )
                                nc.tensor.matmul(
                                    r_ps[0:HD, :],
                                    lhsT=e_sb,
                                    rhs=rcp_r,
                                    start=True,
                                    stop=True,
                                )
                                qj = h // 2
                                if h % 2 == 0:
                                    nc.vector.tensor_mul(
                                        out=outT[0:HD, qj, nsl],
                                        in0=av_sb[0:HD, :],
                                        in1=r_ps[0:HD, :],
                                    )
                                else:
                                    tmp = attn.tile(
                                        [HD, 512], F32R, tag="otmp", bufs=2,
                                        name="tmp",
                                    )
                                    nc.vector.tensor_mul(
                                        out=tmp,
                                        in0=av_sb[0:HD, :],
                                        in1=r_ps[0:HD, :],
                                    )
                                    nc.sync.dma_start(
                                        out=outT[HD:P, qj, nsl], in_=tmp
                                    )

                        return norm

                    pending_norm[0] = make_norm()
                    if nh == 0 and h == 0:
                        # projection weights: one dispatch on the Act ring,
                        # after h0's exps -- off the startup burst entirely
                        nc.sync.dma_start(
                            out=pw_sb, in_=dram_cc(pwT_d, 0, C)
                        )
                    if nh == 1 and h % 3 == 1:
                        # interleave pass-0 projection rows into pass 1
                        proj_chunk(h // 3)
                # (pending norm for h==11 is flushed inside the next pass /
                # tail below)
            # keep the PE clock warm through the final norm latency chain
            with nc.named_scope("warmkeep"):
                ps_w2 = ps_mx.tile([P, 512], F32, tag="mx", name="ps_wk")
                for _ in range(6):
                    nc.tensor.matmul(
                        ps_w2[0:HD, :],
                        lhsT=e_sb,
                        rhs=rcp_rs[0],
                        start=True,
                        stop=True,
                    )
            pending_norm[0]()
            pending_norm[0] = None
            for nch in range(4, NC_):
                proj_chunk(nch)

    nc.compile()
    return nc


_NC_CACHE = None


def _get_nc():
    global _NC_CACHE
    if _NC_CACHE is None:
        _NC_CACHE = _build_nc()
    return _NC_CACHE


def kernel(x, policy, qkv_w, proj_w, proj_b):
    global LAST_RESULTS
    x = np.asarray(x, dtype=np.float32)
    policy = np.asarray(policy, dtype=np.float32)
    qkv_w = np.asarray(qkv_w, dtype=np.float32)
    proj_w = np.asarray(proj_w, dtype=np.float32)
    proj_b = np.asarray(proj_b, dtype=np.float32)

    wqkT = np.ascontiguousarray(qkv_w[: 2 * H * HD].T)  # [768, 1536]
    wvT = np.ascontiguousarray(qkv_w[2 * H * HD :].T)  # [768, 768]
    pwT = np.ascontiguousarray(proj_w.T)  # [768, 768]

    in_maps = []
    for b in range(B):
        pol = policy[b, :, 0]
        polc = np.maximum(pol, 1e-30)
        # [p, chunk] layout: global n = chunk*128 + p
        polT = np.ascontiguousarray(pol.reshape(NC_, P).T)
        dmask = np.ones((P, NC_, P), dtype=np.float32)
        rng = np.arange(P)
        for kch in range(NC_):
            dmask[rng, kch, rng] = 1.0 / polc[kch * P + rng]
        in_maps.append(
            dict(
                xT=np.ascontiguousarray(x[b].T),
                wqkT=wqkT,
                wvT=wvT,
                pwT=pwT,
                bias=proj_b,
                polT=polT.astype(np.float32),
                dmask=dmask,
            )
        )

    nc = _get_nc()
    trace = os.environ.get("KERNEL_TRACE", "0") == "1"
    res = run_bass_kernel_spmd(
        nc,
        in_maps,
        core_ids=list(range(B)),
        trace=trace,
        trace_cores=list(range(B)) if trace else None,
        stitch_traces=False,
    )
    LAST_RESULTS = res
    return np.stack([res.results[b]["y"] for b in range(B)], axis=0)


# revision 28
# speedup vs baseline: 1.1617x; 1.1617x over previous
"""Trainium2 Bass kernel for policy-weighted multi-head attention.

Reference computation (per batch b, 8 batches):
    qkv = x @ qkv_w.T                     # [N, 3*H*HD]
    q, k, v per head                      # H=12 heads, HD=64
    s = (q * HD^-0.5) @ k.T               # [N, N]
    a[n,m] ~ exp(s[n,m]) * (pol[m] + (1-pol[m])*eye)  normalized over m
    out = a @ v ; y = out @ proj_w.T + b

Sharding: pure data parallel, one batch per NeuronCore (8 cores).

Kernel strategy (per core):
  - Host pre-transposes x, qkv_w, proj_w so no on-chip transposes are needed.
  - All matmuls run as float32r (full fp32 data, ~250ns per [128x128x512]).
  - Attention runs in the S^T layout (partitions = key index m): the softmax
    sum over m folds into the PE via an appended ones column on the
    (policy-prescaled) V; the denominator appears as row 64 of the
    attention-output matmul.
  - The policy multiply is folded into V (rows pre-scaled by pol[m]); the
    diagonal term becomes masked multiplies with a precomputed
    [128, 8, 128] mask whose diagonal is 1/pol.
  - nh-major scheduling: the two 512-wide halves of the query axis are
    processed as outer passes over all 12 heads; all 6 q/k pair buffers stay
    resident (prefetched during pass 0), and the projection matmuls for
    pass-0 rows are interleaved into pass 1 so the PE never drains between
    attention and projection.
  - Normalization is DMA-free: per (head, half), 1/denominator via a single
    fast-approx DVE reciprocal on the av row, partition-broadcast by a K=1
    ones-matmul into PSUM, then one DVE multiply into the output tile.  Only
    the odd-head partition shift (rows 64..127) needs an SBUF->SBUF DMA, on
    the otherwise-idle SWDGE ring.  Norm emission is deferred into the next
    head's st loop so the PE never waits on the DVE reciprocal chain.
  - Bulk inputs are single-dispatch DMAs (each dma_start costs ~0.65us of
    sequencer dispatch); the first qk pair + x streams go on the Act HWDGE
    ring, which is idle during the sync ring's kernel preamble, and dummy
    warm-up matmuls run during the load so the PE clock-gate (HAM) reaches
    2.4GHz before real work starts.
  - max-subtraction and the eps terms of the reference softmax are dropped:
    logits are ~N(0,1) so exp() cannot overflow, and the eps corrections
    are ~1e-9 relative -- far below fp32 noise.
"""

import os

os.environ.setdefault("JAX_PLATFORMS", "axon")

from contextlib import ExitStack

import ml_dtypes
import numpy as np

import concourse.bass as bass
import concourse.tile as tile
from concourse import bacc, mybir
from concourse.bass_utils import run_bass_kernel_spmd

B, N, C = 8, 1024, 768
H, HD = 12, 64
SCALE = HD ** (-0.5)
F32 = mybir.dt.float32
F32R = mybir.dt.float32r
BF16 = mybir.dt.bfloat16
P = 128
NC_ = N // P  # 8 seq chunks
CC = C // P  # 6 channel chunks
NH = N // 512  # 2 free-dim halves of the seq axis

LAST_RESULTS = None  # BassKernelResults of the most recent run (for test.py)


def _build_nc():
    nc = bacc.Bacc(None, target_bir_lowering=False)

    xT_d = nc.dram_tensor("xT", [C, N], F32R, kind="ExternalInput")
    wqkT_d = nc.dram_tensor("wqkT", [C, 2 * H * HD], F32R, kind="ExternalInput")
    wvT_d = nc.dram_tensor("wvT", [C, H * HD], F32R, kind="ExternalInput")
    pwT_d = nc.dram_tensor("pwT", [C, C], F32R, kind="ExternalInput")
    bias_d = nc.dram_tensor("bias", [C], F32, kind="ExternalInput")
    polT_d = nc.dram_tensor("polT", [P, NC_], F32, kind="ExternalInput")
    dmask_d = nc.dram_tensor("dmask", [P, NC_, P], F32R, kind="ExternalInput")
    y_d = nc.dram_tensor("y", [N, C], F32, kind="ExternalOutput")

    def dram_cc(t_d, c0, w):
        # [768, w] DRAM slice viewed as [128, CC, w] for one-dispatch loads
        return t_d[:, c0 : c0 + w].rearrange("(cc p) w -> p cc w", p=P)

    with ExitStack() as ctx:
        tc = ctx.enter_context(tile.TileContext(nc))

        persist = ctx.enter_context(tc.tile_pool(name="persist", bufs=1))
        xT_sb = persist.tile([P, CC, N], F32R)
        # v in natural layout, pol-scaled, with a pol column at d=64
        v_aug = persist.tile([P, NC_, H, HD + 1], F32R)
        pw_sb = persist.tile([P, CC, C], F32R)
        b_sb = persist.tile([P, C], F32)
        polT_sb = persist.tile([P, NC_], F32)
        dmask_sb = persist.tile([P, NC_, P], F32R)
        e_f32 = persist.tile([P, HD], F32)
        e_sb = persist.tile([P, HD], F32R)  # unit row 64: selects 1/den
        z_f32 = persist.tile([P, 512], F32)
        rcp_rs = [persist.tile([P, 512], F32R, name=f"rcp_r{i}") for i in (0, 1)]
        outT = persist.tile([P, CC, N], F32R)

        qkp = ctx.enter_context(tc.tile_pool(name="qkp", bufs=1))
        ps_st = ctx.enter_context(tc.tile_pool(name="ps_st", bufs=3, space="PSUM"))
        ps_av = ctx.enter_context(tc.tile_pool(name="ps_av", bufs=1, space="PSUM"))
        ps_mx = ctx.enter_context(tc.tile_pool(name="ps_mx", bufs=1, space="PSUM"))

        nc.vector.memset(e_f32, 0.0)
        nc.vector.memset(e_f32[HD : HD + 1, :], 1.0)
        nc.vector.tensor_copy(out=e_sb, in_=e_f32)
        # rows 65..127 of the broadcast rhs must be real zeros: the K=128
        # matmul streams all 128 partitions on hardware
        nc.vector.memset(z_f32, 0.0)
        for t in rcp_rs:
            nc.vector.tensor_copy(out=t, in_=z_f32)

        # --- priority startup on the Act ring: its sequencer is idle while
        # the sync ring runs the kernel preamble (~7us), and each dma_start
        # costs ~0.65us of sequencer dispatch, so these are one-per-tensor.
        wqk_ts = {0: qkp.tile([P, CC, 2, P], F32R, tag="wqk", bufs=3, name="wqk_t")}
        qk_ts = {}
        for cc in range(CC):
            for kk, j in ((0, 0), (1, CC)):
                nc.sync.dma_start(
                    out=wqk_ts[0][:, cc, kk, :],
                    in_=wqkT_d[cc * P : (cc + 1) * P, j * P : (j + 1) * P],
                )
            nc.sync.dma_start(
                out=xT_sb[:, cc, 0:512], in_=xT_d[cc * P : (cc + 1) * P, 0:512]
            )
        # bulk on the sync ring, in need order
        nc.sync.dma_start(out=polT_sb, in_=polT_d[:])
        for cc in range(CC):
            nc.sync.dma_start(
                out=xT_sb[:, cc, 512:1024],
                in_=xT_d[cc * P : (cc + 1) * P, 512:1024],
            )

        # HAM warm-up: the PE clock-gate defaults to 1.2GHz and only reaches
        # 2.4GHz after ~3.4us of sustained matmul activity.  Burn dummy
        # matmuls on the first-arriving weight tile while the startup burst
        # streams in, so real matmuls run at full clock.
        with nc.named_scope("warmup"):
            ps_w = ps_mx.tile([P, 512], F32, tag="mx", name="ps_warm")
            for _ in range(30):
                nc.tensor.matmul(
                    ps_w[:, 0:128],
                    lhsT=wqk_ts[0][:, 0, 0, :],
                    rhs=wqk_ts[0][:, 0, 0, :],
                    start=True,
                    stop=True,
                )

        def emit_qk_mms(jq, nhs=(0, 1)):
            """qk^T matmuls for pair jq: qk_t[:,0,:] = q chunk jq,
            qk_t[:,1,:] = k chunk jq+6 (embedding dim on partitions)."""
            wqk_t, qk_t = wqk_ts[jq], qk_ts[jq]
            with nc.named_scope("qk_mm"):
                for kk in range(2):
                    for nh in nhs:
                        ps = ps_mx.tile([P, 512], F32, tag="mx", name="ps_qk")
                        for cc in range(CC):
                            nc.tensor.matmul(
                                ps,
                                lhsT=wqk_t[:, cc, kk, :],
                                rhs=xT_sb[:, cc, nh * 512 : (nh + 1) * 512],
                                start=(cc == 0),
                                stop=(cc == CC - 1),
                            )
                        nc.scalar.activation(
                            out=qk_t[:, kk, nh * 512 : (nh + 1) * 512],
                            in_=ps,
                            func=mybir.ActivationFunctionType.Copy,
                        )

        qk_ts[0] = qkp.tile([P, 2, N], BF16, tag="qkT", bufs=CC, name="qk_t")
        emit_qk_mms(0, nhs=(0,))
        # pol columns of v_aug: DVE free-dim broadcast copies
        for nch in range(NC_):
            nc.vector.tensor_copy(
                out=v_aug[:, nch, :, HD : HD + 1],
                in_=polT_sb[:, nch : nch + 1]
                .unsqueeze(1)
                .broadcast_to((P, H, 1)),
            )
        emit_qk_mms(0, nhs=(1,))

        # ---- v natural layout, pol-scaled, into v_aug --------------------
        with tc.tile_pool(name="phv", bufs=1) as phv:
            wv_sb = phv.tile([P, CC, H * HD], F32R)
            for cc in range(CC):
                nc.sync.dma_start(
                    out=wv_sb[:, cc], in_=wvT_d[cc * P : (cc + 1) * P, :]
                )
            nc.sync.dma_start(out=dmask_sb, in_=dmask_d[:])
            nc.sync.dma_start(out=b_sb, in_=bias_d[:].partition_broadcast(P))
            with nc.named_scope("v_mm"):
                for nch in range(NC_):
                    for ev0, ev_sz, h0 in ((0, 512, 0), (512, 256, 8)):
                        nheads = ev_sz // HD
                        ps = ps_st.tile([P, 2, 512], F32, tag="st", name="ps_v")
                        psv = ps[:, 0, :ev_sz]
                        for cc in range(CC):
                            nc.tensor.matmul(
                                psv,
                                lhsT=xT_sb[:, cc, nch * P : (nch + 1) * P],
                                rhs=wv_sb[:, cc, ev0 : ev0 + ev_sz],
                                start=(cc == 0),
                                stop=(cc == CC - 1),
                            )
                        nc.vector.tensor_mul(
                            out=v_aug[:, nch, h0 : h0 + nheads, 0:HD],
                            in0=psv.rearrange("p (h d) -> p h d", d=HD),
                            in1=polT_sb[:, nch : nch + 1]
                            .unsqueeze(1)
                            .broadcast_to((P, nheads, HD)),
                        )

        # ------------------- attention, nh-major --------------------------
        with tc.tile_pool(name="attn", bufs=1) as attn:
            pending_norm = [None]  # deferred norm closure for the prev head

            def proj_chunk(nch):
                with nc.named_scope("proj_mm"):
                    y_t = attn.tile([P, C], F32, tag="y", bufs=2, name="y_t")
                    for oi, (o0, o_sz) in enumerate(((0, 512), (512, 256))):
                        pool, tagn = (ps_mx, "mx") if oi == 0 else (ps_av, "av")
                        ps = pool.tile([P, 512], F32, tag=tagn, name="ps_yt")
                        psy = ps[:, :o_sz]
                        for ec in range(CC):
                            nc.tensor.matmul(
                                psy,
                                lhsT=outT[:, ec, nch * P : (nch + 1) * P],
                                rhs=pw_sb[:, ec, o0 : o0 + o_sz],
                                start=(ec == 0),
                                stop=(ec == CC - 1),
                            )
                        nc.vector.tensor_add(
                            out=y_t[:, o0 : o0 + o_sz],
                            in0=psy,
                            in1=b_sb[:, o0 : o0 + o_sz],
                        )
                    nc.sync.dma_start(out=y_d[nch * P : (nch + 1) * P, :], in_=y_t)

            for nh in range(NH):
                nsl = slice(nh * 512, (nh + 1) * 512)
                for h in range(H):
                    j = h // 2
                    hp = 64 * (h % 2)
                    if nh == 0 and h % 2 == 0 and j + 1 < CC:
                        # prefetch next q/k pair (DMA + matmuls) during pass 0
                        jn = j + 1
                        wqk_ts[jn] = qkp.tile(
                            [P, CC, 2, P], F32R, tag="wqk", bufs=3, name="wqk_t"
                        )
                        for kk, jj in ((0, jn), (1, jn + CC)):
                            for cc in range(CC):
                                nc.sync.dma_start(
                                    out=wqk_ts[jn][:, cc, kk, :],
                                    in_=wqkT_d[
                                        cc * P : (cc + 1) * P,
                                        jj * P : (jj + 1) * P,
                                    ],
                                )
                        qk_ts[jn] = qkp.tile(
                            [P, 2, N], BF16, tag="qkT", bufs=CC, name="qk_t"
                        )
                        emit_qk_mms(jn)
                    qk_pair = qk_ts[j]
                    av = ps_av.tile([HD + 1, 512], F32, tag="av", name="av")
                    E_ts = []
                    # chunk-pair pipeline: st(t) || exp(t-1..2) || av(t-3)
                    for t in range(NC_ // 2 + 3):
                        if t == 3 and pending_norm[0] is not None:
                            # prev head's norm, emitted once this head's st
                            # matmuls are queued so the PE never waits on it
                            pending_norm[0]()
                            pending_norm[0] = None
                        if t < NC_ // 2:
                            st = ps_st.tile([P, 2, 512], F32, tag="st", name="st")
                            E_t = attn.tile(
                                [P, 2, 512], F32R, tag="E", bufs=5, name="E_t"
                            )
                            with nc.named_scope("st_mm"):
                                for k in range(2):
                                    mc = 2 * t + k
                                    nc.tensor.matmul(
                                        st[:, k, :],
                                        lhsT=qk_pair[
                                            hp : hp + HD, 1, mc * P : (mc + 1) * P
                                        ],
                                        rhs=qk_pair[hp : hp + HD, 0, nsl],
                                        start=True,
                                        stop=True,
                                    )
                            nc.scalar.activation(
                                out=E_t,
                                in_=st,
                                func=mybir.ActivationFunctionType.Exp,
                                scale=SCALE,
                            )
                            if t in (2 * nh, 2 * nh + 1):
                                # chunks 2t, 2t+1 hold the diagonal
                                diag_ap = bass.AP(
                                    tensor=E_t.tensor,
                                    offset=E_t.offset + (2 * t * P - 512 * nh),
                                    ap=[E_t.ap[0], [512 + P, 2], [1, P]],
                                )
                                nc.vector.tensor_mul(
                                    out=diag_ap,
                                    in0=diag_ap,
                                    in1=dmask_sb[:, 2 * t : 2 * t + 2, :],
                                )
                            E_ts.append(E_t)
                        if t >= 3:
                            ta = t - 3
                            E_a = E_ts[ta]
                            with nc.named_scope("av_mm"):
                                for k in range(2):
                                    mc = 2 * ta + k
                                    nc.tensor.matmul(
                                        av,
                                        lhsT=v_aug[:, mc, h, :],
                                        rhs=E_a[:, k, :],
                                        start=(mc == 0),
                                        stop=(mc == NC_ - 1),
                                    )
                    av_sb = attn.tile(
                        [HD + 1, 512], F32, tag="avsb", bufs=3, name="av_sb"
                    )
                    nc.vector.tensor_copy(out=av_sb, in_=av)

                    def make_norm(av_sb=av_sb, h=h, nh=nh, nsl=nsl):
                        def norm():
                            with nc.named_scope("norm"):
                                # rcp rows 0..63 are finite filler (the av
                                # values) so the K=65 broadcast matmul below
                                # never multiplies 0 by uninitialized bits
                                rcp = attn.tile(
                                    [HD + 1, 512], F32, tag="rcp", bufs=2,
                                    name="rcp",
                                )
                                # full-tile op from partition 0: the custom
                                # DVE op returns zeros on HW when started at
                                # a partition offset.  Rows 0..63 are 1/av --
                                # finite filler, zeroed by the selector.
                                nc.vector.reciprocal_approx_fast(
                                    out=rcp, in_=av_sb
                                )
                                # round to f32r on the (lightly loaded) Act
                                # engine; the f32r matmul requires it
                                rcp_r = rcp_rs[h % 2]
                                nc.scalar.activation(
                                    out=rcp_r[0 : HD + 1, :],
                                    in_=rcp,
                                    func=mybir.ActivationFunctionType.Copy,
                                )
                                r_ps = ps_mx.tile(
                                    [P, 512], F32, tag="mx", name="r_ps"
                                )
                                nc.tensor.matmul(
                                    r_ps[0:HD, :],
                                    lhsT=e_sb,
                                    rhs=rcp_r,
                                    start=True,
                                    stop=True,
                                )
                                qj = h // 2
                                if h % 2 == 0:
                                    nc.vector.tensor_mul(
                                        out=outT[0:HD, qj, nsl],
                                        in0=av_sb[0:HD, :],
                                        in1=r_ps[0:HD, :],
                                    )
                                else:
                                    tmp = attn.tile(
                                        [HD, 512], F32R, tag="otmp", bufs=2,
                                        name="tmp",
                                    )
                                    nc.vector.tensor_mul(
                                        out=tmp,
                                        in0=av_sb[0:HD, :],
                                        in1=r_ps[0:HD, :],
                                    )
                                    nc.sync.dma_start(
                                        out=outT[HD:P, qj, nsl], in_=tmp
                                    )

                        return norm

                    pending_norm[0] = make_norm()
                    if nh == 0 and h == 0:
                        # projection weights: one dispatch on the Act ring,
                        # after h0's exps -- off the startup burst entirely
                        for cc in range(CC):
                            nc.sync.dma_start(
                                out=pw_sb[:, cc],
                                in_=pwT_d[cc * P : (cc + 1) * P, :],
                            )
                    if nh == 1 and h % 3 == 1:
                        # interleave pass-0 projection rows into pass 1
                        proj_chunk(h // 3)
                # (pending norm for h==11 is flushed inside the next pass /
                # tail below)
            # keep the PE clock warm through the final norm latency chain
            with nc.named_scope("warmkeep"):
                ps_w2 = ps_mx.tile([P, 512], F32, tag="mx", name="ps_wk")
                for _ in range(6):
                    nc.tensor.matmul(
                        ps_w2[0:HD, :],
                        lhsT=e_sb,
                        rhs=rcp_rs[0],
                        start=True,
                        stop=True,
                    )
            pending_norm[0]()
            pending_norm[0] = None
            for nch in range(4, NC_):
                proj_chunk(nch)

    nc.compile()
    return nc


_NC_CACHE = None


def _get_nc():
    global _NC_CACHE
    if _NC_CACHE is None:
        _NC_CACHE = _build_nc()
    return _NC_CACHE


def kernel(x, policy, qkv_w, proj_w, proj_b):
    global LAST_RESULTS
    x = np.asarray(x, dtype=np.float32)
    policy = np.asarray(policy, dtype=np.float32)
    qkv_w = np.asarray(qkv_w, dtype=np.float32)
    proj_w = np.asarray(proj_w, dtype=np.float32)
    proj_b = np.asarray(proj_b, dtype=np.float32)

    wqkT = np.ascontiguousarray(qkv_w[: 2 * H * HD].T)  # [768, 1536]
    wvT = np.ascontiguousarray(qkv_w[2 * H * HD :].T)  # [768, 768]
    pwT = np.ascontiguousarray(proj_w.T)  # [768, 768]

    in_maps = []
    for b in range(B):
        pol = policy[b, :, 0]
        polc = np.maximum(pol, 1e-30)
        # [p, chunk] layout: global n = chunk*128 + p
        polT = np.ascontiguousarray(pol.reshape(NC_, P).T)
        dmask = np.ones((P, NC_, P), dtype=np.float32)
        rng = np.arange(P)
        for kch in range(NC_):
            dmask[rng, kch, rng] = 1.0 / polc[kch * P + rng]
        in_maps.append(
            dict(
                xT=np.ascontiguousarray(x[b].T),
                wqkT=wqkT,
                wvT=wvT,
                pwT=pwT,
                bias=proj_b,
                polT=polT.astype(np.float32),
                dmask=dmask,
            )
        )

    nc = _get_nc()
    trace = os.environ.get("KERNEL_TRACE", "0") == "1"
    res = run_bass_kernel_spmd(
        nc,
        in_maps,
        core_ids=list(range(B)),
        trace=trace,
        trace_cores=list(range(B)) if trace else None,
        stitch_traces=False,
    )
    LAST_RESULTS = res
    return np.stack([res.results[b]["y"] for b in range(B)], axis=0)


# revision 30
# speedup vs baseline: 1.1779x; 1.0139x over previous
"""Trainium2 Bass kernel for policy-weighted multi-head attention.

Reference computation (per batch b, 8 batches):
    qkv = x @ qkv_w.T                     # [N, 3*H*HD]
    q, k, v per head                      # H=12 heads, HD=64
    s = (q * HD^-0.5) @ k.T               # [N, N]
    a[n,m] ~ exp(s[n,m]) * (pol[m] + (1-pol[m])*eye)  normalized over m
    out = a @ v ; y = out @ proj_w.T + b

Sharding: pure data parallel, one batch per NeuronCore (8 cores).

Kernel strategy (per core):
  - Host pre-transposes x, qkv_w, proj_w so no on-chip transposes are needed.
  - All matmuls run as float32r (full fp32 data, ~250ns per [128x128x512]).
  - Attention runs in the S^T layout (partitions = key index m): the softmax
    sum over m folds into the PE via an appended ones column on the
    (policy-prescaled) V; the denominator appears as row 64 of the
    attention-output matmul.
  - The policy multiply is folded into V (rows pre-scaled by pol[m]); the
    diagonal term becomes masked multiplies with a precomputed
    [128, 8, 128] mask whose diagonal is 1/pol.
  - nh-major scheduling: the two 512-wide halves of the query axis are
    processed as outer passes over all 12 heads; all 6 q/k pair buffers stay
    resident (prefetched during pass 0), and the projection matmuls for
    pass-0 rows are interleaved into pass 1 so the PE never drains between
    attention and projection.
  - Normalization is DMA-free: per (head, half), 1/denominator via a single
    fast-approx DVE reciprocal on the av row, partition-broadcast by a K=1
    ones-matmul into PSUM, then one DVE multiply into the output tile.  Only
    the odd-head partition shift (rows 64..127) needs an SBUF->SBUF DMA, on
    the otherwise-idle SWDGE ring.  Norm emission is deferred into the next
    head's st loop so the PE never waits on the DVE reciprocal chain.
  - Bulk inputs are single-dispatch DMAs (each dma_start costs ~0.65us of
    sequencer dispatch); the first qk pair + x streams go on the Act HWDGE
    ring, which is idle during the sync ring's kernel preamble, and dummy
    warm-up matmuls run during the load so the PE clock-gate (HAM) reaches
    2.4GHz before real work starts.
  - max-subtraction and the eps terms of the reference softmax are dropped:
    logits are ~N(0,1) so exp() cannot overflow, and the eps corrections
    are ~1e-9 relative -- far below fp32 noise.
"""

import os

os.environ.setdefault("JAX_PLATFORMS", "axon")

from contextlib import ExitStack

import ml_dtypes
import numpy as np

import concourse.bass as bass
import concourse.tile as tile
from concourse import bacc, mybir
from concourse.bass_utils import run_bass_kernel_spmd

B, N, C = 8, 1024, 768
H, HD = 12, 64
SCALE = HD ** (-0.5)
F32 = mybir.dt.float32
F32R = mybir.dt.float32r
BF16 = mybir.dt.bfloat16
P = 128
NC_ = N // P  # 8 seq chunks
CC = C // P  # 6 channel chunks
NH = N // 512  # 2 free-dim halves of the seq axis

LAST_RESULTS = None  # BassKernelResults of the most recent run (for test.py)


def _build_nc():
    nc = bacc.Bacc(None, target_bir_lowering=False)

    xT_d = nc.dram_tensor("xT", [C, N], F32R, kind="ExternalInput")
    wqkT_d = nc.dram_tensor("wqkT", [C, 2 * H * HD], F32R, kind="ExternalInput")
    wvT_d = nc.dram_tensor("wvT", [C, H * HD], F32R, kind="ExternalInput")
    pwT_d = nc.dram_tensor("pwT", [C, C], F32R, kind="ExternalInput")
    bias_d = nc.dram_tensor("bias", [C], F32, kind="ExternalInput")
    polT_d = nc.dram_tensor("polT", [P, NC_], F32, kind="ExternalInput")
    dmask_d = nc.dram_tensor("dmask", [P, NC_, P], F32R, kind="ExternalInput")
    y_d = nc.dram_tensor("y", [N, C], F32, kind="ExternalOutput")

    def dram_cc(t_d, c0, w):
        # [768, w] DRAM slice viewed as [128, CC, w] for one-dispatch loads
        return t_d[:, c0 : c0 + w].rearrange("(cc p) w -> p cc w", p=P)

    with ExitStack() as ctx:
        tc = ctx.enter_context(tile.TileContext(nc))

        persist = ctx.enter_context(tc.tile_pool(name="persist", bufs=1))
        xT_sb = persist.tile([P, CC, N], F32R)
        # v in natural layout, pol-scaled, with a pol column at d=64
        v_aug = persist.tile([P, NC_, H, HD + 1], F32R)
        pw_sb = persist.tile([P, CC, C], F32R)
        b_sb = persist.tile([P, C], F32)
        polT_sb = persist.tile([P, NC_], F32)
        dmask_sb = persist.tile([P, NC_, P], F32R)
        e_f32 = persist.tile([P, HD], F32)
        e_sb = persist.tile([P, HD], F32R)  # unit row 64: selects 1/den
        z_f32 = persist.tile([P, 512], F32)
        rcp_rs = [persist.tile([P, 512], F32R, name=f"rcp_r{i}") for i in (0, 1)]
        outT = persist.tile([P, CC, N], F32R)

        qkp = ctx.enter_context(tc.tile_pool(name="qkp", bufs=1))
        ps_st = ctx.enter_context(tc.tile_pool(name="ps_st", bufs=3, space="PSUM"))
        ps_av = ctx.enter_context(tc.tile_pool(name="ps_av", bufs=1, space="PSUM"))
        ps_mx = ctx.enter_context(tc.tile_pool(name="ps_mx", bufs=1, space="PSUM"))

        nc.vector.memset(e_f32, 0.0)
        nc.vector.memset(e_f32[HD : HD + 1, :], 1.0)
        nc.vector.tensor_copy(out=e_sb, in_=e_f32)
        # rows 65..127 of the broadcast rhs must be real zeros: the K=128
        # matmul streams all 128 partitions on hardware
        nc.vector.memset(z_f32, 0.0)
        for t in rcp_rs:
            nc.vector.tensor_copy(out=t, in_=z_f32)

        # --- priority startup on the Act ring: its sequencer is idle while
        # the sync ring runs the kernel preamble (~7us), and each dma_start
        # costs ~0.65us of sequencer dispatch, so these are one-per-tensor.
        wqk_ts = {0: qkp.tile([P, CC, 2, P], F32R, tag="wqk", bufs=4, name="wqk_t")}
        qk_ts = {}
        for cc in range(CC):
            for kk, j in ((0, 0), (1, CC)):
                nc.sync.dma_start(
                    out=wqk_ts[0][:, cc, kk, :],
                    in_=wqkT_d[cc * P : (cc + 1) * P, j * P : (j + 1) * P],
                )
            nc.sync.dma_start(
                out=xT_sb[:, cc, 0:512], in_=xT_d[cc * P : (cc + 1) * P, 0:512]
            )
        # bulk on the sync ring, in need order
        nc.sync.dma_start(out=polT_sb, in_=polT_d[:])
        for cc in range(CC):
            nc.sync.dma_start(
                out=xT_sb[:, cc, 512:1024],
                in_=xT_d[cc * P : (cc + 1) * P, 512:1024],
            )

        # HAM warm-up: the PE clock-gate defaults to 1.2GHz and only reaches
        # 2.4GHz after ~3.4us of sustained matmul activity.  Burn dummy
        # matmuls on the first-arriving weight tile while the startup burst
        # streams in, so real matmuls run at full clock.
        with nc.named_scope("warmup"):
            ps_w = ps_mx.tile([P, 512], F32, tag="mx", name="ps_warm")
            for _ in range(30):
                nc.tensor.matmul(
                    ps_w[:, 0:128],
                    lhsT=wqk_ts[0][:, 0, 0, :],
                    rhs=wqk_ts[0][:, 0, 0, :],
                    start=True,
                    stop=True,
                )

        def emit_qk_mms(jq, nhs=(0, 1)):
            """qk^T matmuls for pair jq: qk_t[:,0,:] = q chunk jq,
            qk_t[:,1,:] = k chunk jq+6 (embedding dim on partitions)."""
            wqk_t, qk_t = wqk_ts[jq], qk_ts[jq]
            with nc.named_scope("qk_mm"):
                for kk in range(2):
                    for nh in nhs:
                        ps = ps_mx.tile([P, 512], F32, tag="mx", name="ps_qk")
                        for cc in range(CC):
                            nc.tensor.matmul(
                                ps,
                                lhsT=wqk_t[:, cc, kk, :],
                                rhs=xT_sb[:, cc, nh * 512 : (nh + 1) * 512],
                                start=(cc == 0),
                                stop=(cc == CC - 1),
                            )
                        nc.vector.tensor_copy(
                            out=qk_t[:, kk, nh * 512 : (nh + 1) * 512], in_=ps
                        )

        qk_ts[0] = qkp.tile([P, 2, N], BF16, tag="qkT", bufs=CC, name="qk_t")
        emit_qk_mms(0, nhs=(0,))
        # pol columns of v_aug: DVE free-dim broadcast copies
        for nch in range(NC_):
            nc.vector.tensor_copy(
                out=v_aug[:, nch, :, HD : HD + 1],
                in_=polT_sb[:, nch : nch + 1]
                .unsqueeze(1)
                .broadcast_to((P, H, 1)),
            )
        emit_qk_mms(0, nhs=(1,))

        # ---- v natural layout, pol-scaled, into v_aug --------------------
        with tc.tile_pool(name="phv", bufs=1) as phv:
            wv_sb = phv.tile([P, CC, H * HD], F32R)
            for cc in range(CC):
                nc.sync.dma_start(
                    out=wv_sb[:, cc], in_=wvT_d[cc * P : (cc + 1) * P, :]
                )
            nc.sync.dma_start(out=dmask_sb, in_=dmask_d[:])
            nc.sync.dma_start(out=b_sb, in_=bias_d[:].partition_broadcast(P))
            with nc.named_scope("v_mm"):
                for nch in range(NC_):
                    for ev0, ev_sz, h0 in ((0, 512, 0), (512, 256, 8)):
                        nheads = ev_sz // HD
                        ps = ps_st.tile([P, 2, 512], F32, tag="st", name="ps_v")
                        psv = ps[:, 0, :ev_sz]
                        for cc in range(CC):
                            nc.tensor.matmul(
                                psv,
                                lhsT=xT_sb[:, cc, nch * P : (nch + 1) * P],
                                rhs=wv_sb[:, cc, ev0 : ev0 + ev_sz],
                                start=(cc == 0),
                                stop=(cc == CC - 1),
                            )
                        nc.vector.tensor_mul(
                            out=v_aug[:, nch, h0 : h0 + nheads, 0:HD],
                            in0=psv.rearrange("p (h d) -> p h d", d=HD),
                            in1=polT_sb[:, nch : nch + 1]
                            .unsqueeze(1)
                            .broadcast_to((P, nheads, HD)),
                        )

        # ------------------- attention, nh-major --------------------------
        with tc.tile_pool(name="attn", bufs=1) as attn:
            pending_norm = [None]  # deferred norm closure for the prev head

            def proj_chunk(nch):
                with nc.named_scope("proj_mm"):
                    y_t = attn.tile([P, C], F32, tag="y", bufs=2, name="y_t")
                    for oi, (o0, o_sz) in enumerate(((0, 512), (512, 256))):
                        pool, tagn = (ps_mx, "mx") if oi == 0 else (ps_av, "av")
                        ps = pool.tile([P, 512], F32, tag=tagn, name="ps_yt")
                        psy = ps[:, :o_sz]
                        for ec in range(CC):
                            nc.tensor.matmul(
                                psy,
                                lhsT=outT[:, ec, nch * P : (nch + 1) * P],
                                rhs=pw_sb[:, ec, o0 : o0 + o_sz],
                                start=(ec == 0),
                                stop=(ec == CC - 1),
                            )
                        nc.vector.tensor_add(
                            out=y_t[:, o0 : o0 + o_sz],
                            in0=psy,
                            in1=b_sb[:, o0 : o0 + o_sz],
                        )
                    nc.sync.dma_start(out=y_d[nch * P : (nch + 1) * P, :], in_=y_t)

            for nh in range(NH):
                nsl = slice(nh * 512, (nh + 1) * 512)
                for h in range(H):
                    j = h // 2
                    hp = 64 * (h % 2)
                    if nh == 0 and h % 2 == 0 and j + 1 < CC:
                        # prefetch next q/k pair (DMA + matmuls) during pass 0
                        jn = j + 1
                        wqk_ts[jn] = qkp.tile(
                            [P, CC, 2, P], F32R, tag="wqk", bufs=4, name="wqk_t"
                        )
                        for kk, jj in ((0, jn), (1, jn + CC)):
                            for cc in range(CC):
                                nc.sync.dma_start(
                                    out=wqk_ts[jn][:, cc, kk, :],
                                    in_=wqkT_d[
                                        cc * P : (cc + 1) * P,
                                        jj * P : (jj + 1) * P,
                                    ],
                                )
                        qk_ts[jn] = qkp.tile(
                            [P, 2, N], BF16, tag="qkT", bufs=CC, name="qk_t"
                        )
                        emit_qk_mms(jn)
                    qk_pair = qk_ts[j]
                    av = ps_av.tile([HD + 1, 512], F32, tag="av", name="av")
                    E_ts = []
                    # chunk-pair pipeline: st(t) || exp(t-1..2) || av(t-3)
                    for t in range(NC_ // 2 + 3):
                        if t == 3 and pending_norm[0] is not None:
                            # prev head's norm, emitted once this head's st
                            # matmuls are queued so the PE never waits on it
                            pending_norm[0]()
                            pending_norm[0] = None
                        if t < NC_ // 2:
                            st = ps_st.tile([P, 2, 512], F32, tag="st", name="st")
                            E_t = attn.tile(
                                [P, 2, 512], F32R, tag="E", bufs=5, name="E_t"
                            )
                            with nc.named_scope("st_mm"):
                                for k in range(2):
                                    mc = 2 * t + k
                                    nc.tensor.matmul(
                                        st[:, k, :],
                                        lhsT=qk_pair[
                                            hp : hp + HD, 1, mc * P : (mc + 1) * P
                                        ],
                                        rhs=qk_pair[hp : hp + HD, 0, nsl],
                                        start=True,
                                        stop=True,
                                    )
                            nc.scalar.activation(
                                out=E_t,
                                in_=st,
                                func=mybir.ActivationFunctionType.Exp,
                                scale=SCALE,
                            )
                            if t in (2 * nh, 2 * nh + 1):
                                # chunks 2t, 2t+1 hold the diagonal
                                diag_ap = bass.AP(
                                    tensor=E_t.tensor,
                                    offset=E_t.offset + (2 * t * P - 512 * nh),
                                    ap=[E_t.ap[0], [512 + P, 2], [1, P]],
                                )
                                nc.vector.tensor_mul(
                                    out=diag_ap,
                                    in0=diag_ap,
                                    in1=dmask_sb[:, 2 * t : 2 * t + 2, :],
                                )
                            E_ts.append(E_t)
                        if t >= 3:
                            ta = t - 3
                            E_a = E_ts[ta]
                            with nc.named_scope("av_mm"):
                                for k in range(2):
                                    mc = 2 * ta + k
                                    nc.tensor.matmul(
                                        av,
                                        lhsT=v_aug[:, mc, h, :],
                                        rhs=E_a[:, k, :],
                                        start=(mc == 0),
                                        stop=(mc == NC_ - 1),
                                    )
                    av_sb = attn.tile(
                        [HD + 1, 512], F32, tag="avsb", bufs=3, name="av_sb"
                    )
                    nc.vector.tensor_copy(out=av_sb, in_=av)

                    def make_norm(av_sb=av_sb, h=h, nh=nh, nsl=nsl):
                        def norm():
                            with nc.named_scope("norm"):
                                # rcp rows 0..63 are finite filler (the av
                                # values) so the K=65 broadcast matmul below
                                # never multiplies 0 by uninitialized bits
                                rcp = attn.tile(
                                    [HD + 1, 512], F32, tag="rcp", bufs=2,
                                    name="rcp",
                                )
                                # full-tile op from partition 0: the custom
                                # DVE op returns zeros on HW when started at
                                # a partition offset.  Rows 0..63 are 1/av --
                                # finite filler, zeroed by the selector.
                                nc.vector.reciprocal_approx_fast(
                                    out=rcp, in_=av_sb
                                )
                                # round to f32r on the (lightly loaded) Act
                                # engine; the f32r matmul requires it
                                rcp_r = rcp_rs[h % 2]
                                nc.scalar.activation(
                                    out=rcp_r[0 : HD + 1, :],
                                    in_=rcp,
                                    func=mybir.ActivationFunctionType.Copy,
                                )
                                r_ps = ps_mx.tile(
                                    [P, 512], F32, tag="mx", name="r_ps"
                                )
                                nc.tensor.matmul(
                                    r_ps[0:HD, :],
                                    lhsT=e_sb,
                                    rhs=rcp_r,
                                    start=True,
                                    stop=True,
                                )
                                qj = h // 2
                                if h % 2 == 0:
                                    nc.vector.tensor_mul(
                                        out=outT[0:HD, qj, nsl],
                                        in0=av_sb[0:HD, :],
                                        in1=r_ps[0:HD, :],
                                    )
                                else:
                                    tmp = attn.tile(
                                        [HD, 512], F32R, tag="otmp", bufs=2,
                                        name="tmp",
                                    )
                                    nc.vector.tensor_mul(
                                        out=tmp,
                                        in0=av_sb[0:HD, :],
                                        in1=r_ps[0:HD, :],
                                    )
                                    nc.sync.dma_start(
                                        out=outT[HD:P, qj, nsl], in_=tmp
                                    )

                        return norm

                    pending_norm[0] = make_norm()
                    if nh == 0 and h == 0:
                        # projection weights: one dispatch on the Act ring,
                        # after h0's exps -- off the startup burst entirely
                        for cc in range(CC):
                            nc.sync.dma_start(
                                out=pw_sb[:, cc],
                                in_=pwT_d[cc * P : (cc + 1) * P, :],
                            )
                    if nh == 1 and h % 3 == 1:
                        # interleave pass-0 projection rows into pass 1
                        proj_chunk(h // 3)
                # (pending norm for h==11 is flushed inside the next pass /
                # tail below)
            # keep the PE clock warm through the final norm latency chain
            with nc.named_scope("warmkeep"):
                ps_w2 = ps_mx.tile([P, 512], F32, tag="mx", name="ps_wk")
                for _ in range(6):
                    nc.tensor.matmul(
                        ps_w2[0:HD, :],
                        lhsT=e_sb,
                        rhs=rcp_rs[0],
                        start=True,
                        stop=True,
                    )
            pending_norm[0]()
            pending_norm[0] = None
            for nch in range(4, NC_):
                proj_chunk(nch)

    nc.compile()
    return nc


_NC_CACHE = None


def _get_nc():
    global _NC_CACHE
    if _NC_CACHE is None:
        _NC_CACHE = _build_nc()
    return _NC_CACHE


def kernel(x, policy, qkv_w, proj_w, proj_b):
    global LAST_RESULTS
    x = np.asarray(x, dtype=np.float32)
    policy = np.asarray(policy, dtype=np.float32)
    qkv_w = np.asarray(qkv_w, dtype=np.float32)
    proj_w = np.asarray(proj_w, dtype=np.float32)
    proj_b = np.asarray(proj_b, dtype=np.float32)

    wqkT = np.ascontiguousarray(qkv_w[: 2 * H * HD].T)  # [768, 1536]
    wvT = np.ascontiguousarray(qkv_w[2 * H * HD :].T)  # [768, 768]
    pwT = np.ascontiguousarray(proj_w.T)  # [768, 768]

    in_maps = []
    for b in range(B):
        pol = policy[b, :, 0]
        polc = np.maximum(pol, 1e-30)
        # [p, chunk] layout: global n = chunk*128 + p
        polT = np.ascontiguousarray(pol.reshape(NC_, P).T)
        dmask = np.ones((P, NC_, P), dtype=np.float32)
        rng = np.arange(P)
        for kch in range(NC_):
            dmask[rng, kch, rng] = 1.0 / polc[kch * P + rng]
        in_maps.append(
            dict(
                xT=np.ascontiguousarray(x[b].T),
                wqkT=wqkT,
                wvT=wvT,
                pwT=pwT,
                bias=proj_b,
                polT=polT.astype(np.float32),
                dmask=dmask,
            )
        )

    nc = _get_nc()
    trace = os.environ.get("KERNEL_TRACE", "0") == "1"
    res = run_bass_kernel_spmd(
        nc,
        in_maps,
        core_ids=list(range(B)),
        trace=trace,
        trace_cores=list(range(B)) if trace else None,
        stitch_traces=False,
    )
    LAST_RESULTS = res
    return np.stack([res.results[b]["y"] for b in range(B)], axis=0)


# revision 31
# speedup vs baseline: 1.2068x; 1.0246x over previous
"""Trainium2 Bass kernel for policy-weighted multi-head attention.

Reference computation (per batch b, 8 batches):
    qkv = x @ qkv_w.T                     # [N, 3*H*HD]
    q, k, v per head                      # H=12 heads, HD=64
    s = (q * HD^-0.5) @ k.T               # [N, N]
    a[n,m] ~ exp(s[n,m]) * (pol[m] + (1-pol[m])*eye)  normalized over m
    out = a @ v ; y = out @ proj_w.T + b

Sharding: pure data parallel, one batch per NeuronCore (8 cores).

Kernel strategy (per core):
  - Host pre-transposes x, qkv_w, proj_w so no on-chip transposes are needed.
  - All matmuls run as float32r (full fp32 data, ~250ns per [128x128x512]).
  - Attention runs in the S^T layout (partitions = key index m): the softmax
    sum over m folds into the PE via an appended ones column on the
    (policy-prescaled) V; the denominator appears as row 64 of the
    attention-output matmul.
  - The policy multiply is folded into V (rows pre-scaled by pol[m]); the
    diagonal term becomes masked multiplies with a precomputed
    [128, 8, 128] mask whose diagonal is 1/pol.
  - nh-major scheduling: the two 512-wide halves of the query axis are
    processed as outer passes over all 12 heads; all 6 q/k pair buffers stay
    resident (prefetched during pass 0), and the projection matmuls for
    pass-0 rows are interleaved into pass 1 so the PE never drains between
    attention and projection.
  - Normalization is DMA-free: per (head, half), 1/denominator via a single
    fast-approx DVE reciprocal on the av row, partition-broadcast by a K=1
    ones-matmul into PSUM, then one DVE multiply into the output tile.  Only
    the odd-head partition shift (rows 64..127) needs an SBUF->SBUF DMA, on
    the otherwise-idle SWDGE ring.  Norm emission is deferred into the next
    head's st loop so the PE never waits on the DVE reciprocal chain.
  - Bulk inputs are single-dispatch DMAs (each dma_start costs ~0.65us of
    sequencer dispatch); the first qk pair + x streams go on the Act HWDGE
    ring, which is idle during the sync ring's kernel preamble, and dummy
    warm-up matmuls run during the load so the PE clock-gate (HAM) reaches
    2.4GHz before real work starts.
  - max-subtraction and the eps terms of the reference softmax are dropped:
    logits are ~N(0,1) so exp() cannot overflow, and the eps corrections
    are ~1e-9 relative -- far below fp32 noise.
"""

import os

os.environ.setdefault("JAX_PLATFORMS", "axon")

from contextlib import ExitStack

import ml_dtypes
import numpy as np

import concourse.bass as bass
import concourse.tile as tile
from concourse import bacc, mybir
from concourse.bass_utils import run_bass_kernel_spmd

B, N, C = 8, 1024, 768
H, HD = 12, 64
SCALE = HD ** (-0.5)
F32 = mybir.dt.float32
F32R = mybir.dt.float32r
BF16 = mybir.dt.bfloat16
P = 128
NC_ = N // P  # 8 seq chunks
CC = C // P  # 6 channel chunks
NH = N // 512  # 2 free-dim halves of the seq axis

LAST_RESULTS = None  # BassKernelResults of the most recent run (for test.py)


def _build_nc():
    nc = bacc.Bacc(None, target_bir_lowering=False)

    xT_d = nc.dram_tensor("xT", [C, N], F32R, kind="ExternalInput")
    wqkT_d = nc.dram_tensor("wqkT", [C, 2 * H * HD], F32R, kind="ExternalInput")
    wvT_d = nc.dram_tensor("wvT", [C, H * HD], F32R, kind="ExternalInput")
    pwT_d = nc.dram_tensor("pwT", [C, C], F32R, kind="ExternalInput")
    bias_d = nc.dram_tensor("bias", [C], F32, kind="ExternalInput")
    polT_d = nc.dram_tensor("polT", [P, NC_], F32, kind="ExternalInput")
    dmask_d = nc.dram_tensor("dmask", [P, NC_, P], F32, kind="ExternalInput")
    y_d = nc.dram_tensor("y", [N, C], F32, kind="ExternalOutput")

    def dram_cc(t_d, c0, w):
        # [768, w] DRAM slice viewed as [128, CC, w] for one-dispatch loads
        return t_d[:, c0 : c0 + w].rearrange("(cc p) w -> p cc w", p=P)

    with ExitStack() as ctx:
        tc = ctx.enter_context(tile.TileContext(nc))

        persist = ctx.enter_context(tc.tile_pool(name="persist", bufs=1))
        xT_sb = persist.tile([P, CC, N], F32R)
        # v in natural layout, pol-scaled, with a pol column at d=64
        # (bf16, padded to 66 cols so the per-head stride stays 4B-aligned)
        v_aug = persist.tile([P, NC_, H, HD + 2], BF16)
        pw_sb = persist.tile([P, CC, C], F32R)
        b_sb = persist.tile([P, C], F32)
        polT_sb = persist.tile([P, NC_], F32)
        dmask_sb = persist.tile([P, NC_, P], F32)
        e_f32 = persist.tile([P, HD], F32)
        e_sb = persist.tile([P, HD], F32R)  # unit row 64: selects 1/den
        z_f32 = persist.tile([P, 512], F32)
        rcp_rs = [persist.tile([P, 512], F32R, name=f"rcp_r{i}") for i in (0, 1)]
        outT = persist.tile([P, CC, N], F32R)

        qkp = ctx.enter_context(tc.tile_pool(name="qkp", bufs=1))
        ps_st = ctx.enter_context(tc.tile_pool(name="ps_st", bufs=3, space="PSUM"))
        ps_av = ctx.enter_context(tc.tile_pool(name="ps_av", bufs=1, space="PSUM"))
        ps_mx = ctx.enter_context(tc.tile_pool(name="ps_mx", bufs=1, space="PSUM"))

        nc.vector.memset(e_f32, 0.0)
        nc.vector.memset(e_f32[HD : HD + 1, :], 1.0)
        nc.vector.tensor_copy(out=e_sb, in_=e_f32)
        # rows 65..127 of the broadcast rhs must be real zeros: the K=128
        # matmul streams all 128 partitions on hardware
        nc.vector.memset(z_f32, 0.0)
        for t in rcp_rs:
            nc.vector.tensor_copy(out=t, in_=z_f32)

        # --- priority startup on the Act ring: its sequencer is idle while
        # the sync ring runs the kernel preamble (~7us), and each dma_start
        # costs ~0.65us of sequencer dispatch, so these are one-per-tensor.
        wqk_ts = {0: qkp.tile([P, CC, 2, P], F32R, tag="wqk", bufs=4, name="wqk_t")}
        qk_ts = {}
        for cc in range(CC):
            for kk, j in ((0, 0), (1, CC)):
                nc.sync.dma_start(
                    out=wqk_ts[0][:, cc, kk, :],
                    in_=wqkT_d[cc * P : (cc + 1) * P, j * P : (j + 1) * P],
                )
            nc.sync.dma_start(
                out=xT_sb[:, cc, 0:512], in_=xT_d[cc * P : (cc + 1) * P, 0:512]
            )
        # bulk on the sync ring, in need order
        nc.sync.dma_start(out=polT_sb, in_=polT_d[:])
        for cc in range(CC):
            nc.sync.dma_start(
                out=xT_sb[:, cc, 512:1024],
                in_=xT_d[cc * P : (cc + 1) * P, 512:1024],
            )

        # HAM warm-up: the PE clock-gate defaults to 1.2GHz and only reaches
        # 2.4GHz after ~3.4us of sustained matmul activity.  Burn dummy
        # matmuls on the first-arriving weight tile while the startup burst
        # streams in, so real matmuls run at full clock.
        with nc.named_scope("warmup"):
            ps_w = ps_mx.tile([P, 512], F32, tag="mx", name="ps_warm")
            for _ in range(30):
                nc.tensor.matmul(
                    ps_w[:, 0:128],
                    lhsT=wqk_ts[0][:, 0, 0, :],
                    rhs=wqk_ts[0][:, 0, 0, :],
                    start=True,
                    stop=True,
                )

        def emit_qk_mms(jq, nhs=(0, 1)):
            """qk^T matmuls for pair jq: qk_t[:,0,:] = q chunk jq,
            qk_t[:,1,:] = k chunk jq+6 (embedding dim on partitions)."""
            wqk_t, qk_t = wqk_ts[jq], qk_ts[jq]
            with nc.named_scope("qk_mm"):
                for kk in range(2):
                    for nh in nhs:
                        ps = ps_mx.tile([P, 512], F32, tag="mx", name="ps_qk")
                        for cc in range(CC):
                            nc.tensor.matmul(
                                ps,
                                lhsT=wqk_t[:, cc, kk, :],
                                rhs=xT_sb[:, cc, nh * 512 : (nh + 1) * 512],
                                start=(cc == 0),
                                stop=(cc == CC - 1),
                            )
                        nc.vector.tensor_copy(
                            out=qk_t[:, kk, nh * 512 : (nh + 1) * 512], in_=ps
                        )

        qk_ts[0] = qkp.tile([P, 2, N], BF16, tag="qkT", bufs=CC, name="qk_t")
        emit_qk_mms(0, nhs=(0,))
        # pol columns of v_aug: DVE free-dim broadcast copies
        for nch in range(NC_):
            nc.vector.tensor_copy(
                out=v_aug[:, nch, :, HD : HD + 1],
                in_=polT_sb[:, nch : nch + 1]
                .unsqueeze(1)
                .broadcast_to((P, H, 1)),
            )
        emit_qk_mms(0, nhs=(1,))

        # ---- v natural layout, pol-scaled, into v_aug --------------------
        with tc.tile_pool(name="phv", bufs=1) as phv:
            wv_sb = phv.tile([P, CC, H * HD], F32R)
            for cc in range(CC):
                nc.sync.dma_start(
                    out=wv_sb[:, cc], in_=wvT_d[cc * P : (cc + 1) * P, :]
                )
            nc.sync.dma_start(out=dmask_sb, in_=dmask_d[:])
            nc.sync.dma_start(out=b_sb, in_=bias_d[:].partition_broadcast(P))
            with nc.named_scope("v_mm"):
                for nch in range(NC_):
                    for ev0, ev_sz, h0 in ((0, 512, 0), (512, 256, 8)):
                        nheads = ev_sz // HD
                        ps = ps_st.tile([P, 2, 512], F32, tag="st", name="ps_v")
                        psv = ps[:, 0, :ev_sz]
                        for cc in range(CC):
                            nc.tensor.matmul(
                                psv,
                                lhsT=xT_sb[:, cc, nch * P : (nch + 1) * P],
                                rhs=wv_sb[:, cc, ev0 : ev0 + ev_sz],
                                start=(cc == 0),
                                stop=(cc == CC - 1),
                            )
                        nc.vector.tensor_mul(
                            out=v_aug[:, nch, h0 : h0 + nheads, 0:HD],
                            in0=psv.rearrange("p (h d) -> p h d", d=HD),
                            in1=polT_sb[:, nch : nch + 1]
                            .unsqueeze(1)
                            .broadcast_to((P, nheads, HD)),
                        )

        # ------------------- attention, nh-major --------------------------
        with tc.tile_pool(name="attn", bufs=1) as attn:
            pending_norm = [None]  # deferred norm closure for the prev head

            def proj_chunk(nch):
                with nc.named_scope("proj_mm"):
                    y_t = attn.tile([P, C], F32, tag="y", bufs=2, name="y_t")
                    for oi, (o0, o_sz) in enumerate(((0, 512), (512, 256))):
                        pool, tagn = (ps_mx, "mx") if oi == 0 else (ps_av, "av")
                        ps = pool.tile([P, 512], F32, tag=tagn, name="ps_yt")
                        psy = ps[:, :o_sz]
                        for ec in range(CC):
                            nc.tensor.matmul(
                                psy,
                                lhsT=outT[:, ec, nch * P : (nch + 1) * P],
                                rhs=pw_sb[:, ec, o0 : o0 + o_sz],
                                start=(ec == 0),
                                stop=(ec == CC - 1),
                            )
                        nc.vector.tensor_add(
                            out=y_t[:, o0 : o0 + o_sz],
                            in0=psy,
                            in1=b_sb[:, o0 : o0 + o_sz],
                        )
                    nc.sync.dma_start(out=y_d[nch * P : (nch + 1) * P, :], in_=y_t)

            for nh in range(NH):
                nsl = slice(nh * 512, (nh + 1) * 512)
                for h in range(H):
                    j = h // 2
                    hp = 64 * (h % 2)
                    if nh == 0 and h % 2 == 0 and j + 1 < CC:
                        # prefetch next q/k pair (DMA + matmuls) during pass 0
                        jn = j + 1
                        wqk_ts[jn] = qkp.tile(
                            [P, CC, 2, P], F32R, tag="wqk", bufs=4, name="wqk_t"
                        )
                        for kk, jj in ((0, jn), (1, jn + CC)):
                            for cc in range(CC):
                                nc.sync.dma_start(
                                    out=wqk_ts[jn][:, cc, kk, :],
                                    in_=wqkT_d[
                                        cc * P : (cc + 1) * P,
                                        jj * P : (jj + 1) * P,
                                    ],
                                )
                        qk_ts[jn] = qkp.tile(
                            [P, 2, N], BF16, tag="qkT", bufs=CC, name="qk_t"
                        )
                        emit_qk_mms(jn)
                    qk_pair = qk_ts[j]
                    av = ps_av.tile([HD + 1, 512], F32, tag="av", name="av")
                    E_ts = []
                    # chunk-pair pipeline: st(t) || exp(t-1..2) || av(t-3)
                    for t in range(NC_ // 2 + 3):
                        if t == 3 and pending_norm[0] is not None:
                            # prev head's norm, emitted once this head's st
                            # matmuls are queued so the PE never waits on it
                            pending_norm[0]()
                            pending_norm[0] = None
                        if t < NC_ // 2:
                            st = ps_st.tile([P, 2, 512], F32, tag="st", name="st")
                            E_t = attn.tile(
                                [P, 2, 512], BF16, tag="E", bufs=5, name="E_t"
                            )
                            with nc.named_scope("st_mm"):
                                for k in range(2):
                                    mc = 2 * t + k
                                    nc.tensor.matmul(
                                        st[:, k, :],
                                        lhsT=qk_pair[
                                            hp : hp + HD, 1, mc * P : (mc + 1) * P
                                        ],
                                        rhs=qk_pair[hp : hp + HD, 0, nsl],
                                        start=True,
                                        stop=True,
                                    )
                            nc.scalar.activation(
                                out=E_t,
                                in_=st,
                                func=mybir.ActivationFunctionType.Exp,
                                scale=SCALE,
                            )
                            if t in (2 * nh, 2 * nh + 1):
                                # chunks 2t, 2t+1 hold the diagonal
                                diag_ap = bass.AP(
                                    tensor=E_t.tensor,
                                    offset=E_t.offset + (2 * t * P - 512 * nh),
                                    ap=[E_t.ap[0], [512 + P, 2], [1, P]],
                                )
                                nc.vector.tensor_mul(
                                    out=diag_ap,
                                    in0=diag_ap,
                                    in1=dmask_sb[:, 2 * t : 2 * t + 2, :],
                                )
                            E_ts.append(E_t)
                        if t >= 3:
                            ta = t - 3
                            E_a = E_ts[ta]
                            with nc.named_scope("av_mm"):
                                for k in range(2):
                                    mc = 2 * ta + k
                                    nc.tensor.matmul(
                                        av,
                                        lhsT=v_aug[:, mc, h, 0 : HD + 1],
                                        rhs=E_a[:, k, :],
                                        start=(mc == 0),
                                        stop=(mc == NC_ - 1),
                                    )
                    av_sb = attn.tile(
                        [HD + 1, 512], F32, tag="avsb", bufs=3, name="av_sb"
                    )
                    nc.vector.tensor_copy(out=av_sb, in_=av)

                    def make_norm(av_sb=av_sb, h=h, nh=nh, nsl=nsl):
                        def norm():
                            with nc.named_scope("norm"):
                                # rcp rows 0..63 are finite filler (the av
                                # values) so the K=65 broadcast matmul below
                                # never multiplies 0 by uninitialized bits
                                rcp = attn.tile(
                                    [HD + 1, 512], F32, tag="rcp", bufs=2,
                                    name="rcp",
                                )
                                # full-tile op from partition 0: the custom
                                # DVE op returns zeros on HW when started at
                                # a partition offset.  Rows 0..63 are 1/av --
                                # finite filler, zeroed by the selector.
                                nc.vector.reciprocal_approx_fast(
                                    out=rcp, in_=av_sb
                                )
                                # round to f32r on the (lightly loaded) Act
                                # engine; the f32r matmul requires it
                                rcp_r = rcp_rs[h % 2]
                                nc.scalar.activation(
                                    out=rcp_r[0 : HD + 1, :],
                                    in_=rcp,
                                    func=mybir.ActivationFunctionType.Copy,
                                )
                                r_ps = ps_mx.tile(
                                    [P, 512], F32, tag="mx", name="r_ps"
                                )
                                nc.tensor.matmul(
                                    r_ps[0:HD, :],
                                    lhsT=e_sb,
                                    rhs=rcp_r,
                                    start=True,
                                    stop=True,
                                )
                                qj = h // 2
                                if h % 2 == 0:
                                    nc.vector.tensor_mul(
                                        out=outT[0:HD, qj, nsl],
                                        in0=av_sb[0:HD, :],
                                        in1=r_ps[0:HD, :],
                                    )
                                else:
                                    tmp = attn.tile(
                                        [HD, 512], F32R, tag="otmp", bufs=2,
                                        name="tmp",
                                    )
                                    nc.vector.tensor_mul(
                                        out=tmp,
                                        in0=av_sb[0:HD, :],
                                        in1=r_ps[0:HD, :],
                                    )
                                    nc.sync.dma_start(
                                        out=outT[HD:P, qj, nsl], in_=tmp
                                    )

                        return norm

                    pending_norm[0] = make_norm()
                    if nh == 0 and h == 0:
                        # projection weights: one dispatch on the Act ring,
                        # after h0's exps -- off the startup burst entirely
                        for cc in range(CC):
                            nc.sync.dma_start(
                                out=pw_sb[:, cc],
                                in_=pwT_d[cc * P : (cc + 1) * P, :],
                            )
                    if nh == 1 and h % 3 == 1:
                        # interleave pass-0 projection rows into pass 1
                        proj_chunk(h // 3)
                # (pending norm for h==11 is flushed inside the next pass /
                # tail below)
            # keep the PE clock warm through the final norm latency chain
            with nc.named_scope("warmkeep"):
                ps_w2 = ps_mx.tile([P, 512], F32, tag="mx", name="ps_wk")
                for _ in range(6):
                    nc.tensor.matmul(
                        ps_w2[0:HD, :],
                        lhsT=e_sb,
                        rhs=rcp_rs[0],
                        start=True,
                        stop=True,
                    )
            pending_norm[0]()
            pending_norm[0] = None
            for nch in range(4, NC_):
                proj_chunk(nch)

    nc.compile()
    return nc


_NC_CACHE = None


def _get_nc():
    global _NC_CACHE
    if _NC_CACHE is None:
        _NC_CACHE = _build_nc()
    return _NC_CACHE


def kernel(x, policy, qkv_w, proj_w, proj_b):
    global LAST_RESULTS
    x = np.asarray(x, dtype=np.float32)
    policy = np.asarray(policy, dtype=np.float32)
    qkv_w = np.asarray(qkv_w, dtype=np.float32)
    proj_w = np.asarray(proj_w, dtype=np.float32)
    proj_b = np.asarray(proj_b, dtype=np.float32)

    wqkT = np.ascontiguousarray(qkv_w[: 2 * H * HD].T)  # [768, 1536]
    wvT = np.ascontiguousarray(qkv_w[2 * H * HD :].T)  # [768, 768]
    pwT = np.ascontiguousarray(proj_w.T)  # [768, 768]

    in_maps = []
    for b in range(B):
        pol = policy[b, :, 0]
        polc = np.maximum(pol, 1e-30)
        # [p, chunk] layout: global n = chunk*128 + p
        polT = np.ascontiguousarray(pol.reshape(NC_, P).T)
        dmask = np.ones((P, NC_, P), dtype=np.float32)
        rng = np.arange(P)
        for kch in range(NC_):
            dmask[rng, kch, rng] = 1.0 / polc[kch * P + rng]
        in_maps.append(
            dict(
                xT=np.ascontiguousarray(x[b].T),
                wqkT=wqkT,
                wvT=wvT,
                pwT=pwT,
                bias=proj_b,
                polT=polT.astype(np.float32),
                dmask=dmask,
            )
        )

    nc = _get_nc()
    trace = os.environ.get("KERNEL_TRACE", "0") == "1"
    res = run_bass_kernel_spmd(
        nc,
        in_maps,
        core_ids=list(range(B)),
        trace=trace,
        trace_cores=list(range(B)) if trace else None,
        stitch_traces=False,
    )
    LAST_RESULTS = res
    return np.stack([res.results[b]["y"] for b in range(B)], axis=0)


# revision 33
# speedup vs baseline: 1.2227x; 1.0131x over previous
"""Trainium2 Bass kernel for policy-weighted multi-head attention.

Reference computation (per batch b, 8 batches):
    qkv = x @ qkv_w.T                     # [N, 3*H*HD]
    q, k, v per head                      # H=12 heads, HD=64
    s = (q * HD^-0.5) @ k.T               # [N, N]
    a[n,m] ~ exp(s[n,m]) * (pol[m] + (1-pol[m])*eye)  normalized over m
    out = a @ v ; y = out @ proj_w.T + b

Sharding: pure data parallel, one batch per NeuronCore (8 cores).

Kernel strategy (per core):
  - Host pre-transposes x, qkv_w, proj_w so no on-chip transposes are needed.
  - All matmuls run as float32r (full fp32 data, ~250ns per [128x128x512]).
  - Attention runs in the S^T layout (partitions = key index m): the softmax
    sum over m folds into the PE via an appended ones column on the
    (policy-prescaled) V; the denominator appears as row 64 of the
    attention-output matmul.
  - The policy multiply is folded into V (rows pre-scaled by pol[m]); the
    diagonal term becomes masked multiplies with a precomputed
    [128, 8, 128] mask whose diagonal is 1/pol.
  - nh-major scheduling: the two 512-wide halves of the query axis are
    processed as outer passes over all 12 heads; all 6 q/k pair buffers stay
    resident (prefetched during pass 0), and the projection matmuls for
    pass-0 rows are interleaved into pass 1 so the PE never drains between
    attention and projection.
  - Normalization is DMA-free: per (head, half), 1/denominator via a single
    fast-approx DVE reciprocal on the av row, partition-broadcast by a K=1
    ones-matmul into PSUM, then one DVE multiply into the output tile.  Only
    the odd-head partition shift (rows 64..127) needs an SBUF->SBUF DMA, on
    the otherwise-idle SWDGE ring.  Norm emission is deferred into the next
    head's st loop so the PE never waits on the DVE reciprocal chain.
  - Bulk inputs are single-dispatch DMAs (each dma_start costs ~0.65us of
    sequencer dispatch); the first qk pair + x streams go on the Act HWDGE
    ring, which is idle during the sync ring's kernel preamble, and dummy
    warm-up matmuls run during the load so the PE clock-gate (HAM) reaches
    2.4GHz before real work starts.
  - max-subtraction and the eps terms of the reference softmax are dropped:
    logits are ~N(0,1) so exp() cannot overflow, and the eps corrections
    are ~1e-9 relative -- far below fp32 noise.
"""

import os

os.environ.setdefault("JAX_PLATFORMS", "axon")

from contextlib import ExitStack

import ml_dtypes
import numpy as np

import concourse.bass as bass
import concourse.tile as tile
from concourse import bacc, mybir
from concourse.bass_utils import run_bass_kernel_spmd

B, N, C = 8, 1024, 768
H, HD = 12, 64
SCALE = HD ** (-0.5)
F32 = mybir.dt.float32
F32R = mybir.dt.float32r
BF16 = mybir.dt.bfloat16
P = 128
NC_ = N // P  # 8 seq chunks
CC = C // P  # 6 channel chunks
NH = N // 512  # 2 free-dim halves of the seq axis

LAST_RESULTS = None  # BassKernelResults of the most recent run (for test.py)


def _build_nc():
    nc = bacc.Bacc(None, target_bir_lowering=False)

    xT_d = nc.dram_tensor("xT", [C, N], F32R, kind="ExternalInput")
    wqkT_d = nc.dram_tensor("wqkT", [C, 2 * H * HD], F32R, kind="ExternalInput")
    wvT_d = nc.dram_tensor("wvT", [C, H * HD], F32R, kind="ExternalInput")
    pwT_d = nc.dram_tensor("pwT", [C, C], F32R, kind="ExternalInput")
    bias_d = nc.dram_tensor("bias", [C], F32, kind="ExternalInput")
    polT_d = nc.dram_tensor("polT", [P, NC_], F32, kind="ExternalInput")
    dmask_d = nc.dram_tensor("dmask", [P, NC_, P], F32, kind="ExternalInput")
    y_d = nc.dram_tensor("y", [N, C], F32, kind="ExternalOutput")

    def dram_cc(t_d, c0, w):
        # [768, w] DRAM slice viewed as [128, CC, w] for one-dispatch loads
        return t_d[:, c0 : c0 + w].rearrange("(cc p) w -> p cc w", p=P)

    with ExitStack() as ctx:
        tc = ctx.enter_context(tile.TileContext(nc))

        persist = ctx.enter_context(tc.tile_pool(name="persist", bufs=1))
        xT_sb = persist.tile([P, CC, N], F32R)
        # v in natural layout, pol-scaled, with a pol column at d=64
        # (bf16, padded to 66 cols so the per-head stride stays 4B-aligned)
        v_aug = persist.tile([P, NC_, H, HD + 2], BF16)
        pw_sb = persist.tile([P, CC, C], F32R)
        b_sb = persist.tile([P, C], F32)
        polT_sb = persist.tile([P, NC_], F32)
        dmask_sb = persist.tile([P, NC_, P], F32)
        e_f32 = persist.tile([P, HD], F32)
        e_sb = persist.tile([P, HD], F32R)  # unit row 64: selects 1/den
        z_f32 = persist.tile([P, 512], F32)
        rcp_rs = [persist.tile([P, 512], F32R, name=f"rcp_r{i}") for i in (0, 1)]
        outT = persist.tile([P, CC, N], BF16)
        pw_bf = persist.tile([P, CC, C], BF16)

        qkp = ctx.enter_context(tc.tile_pool(name="qkp", bufs=1))
        ps_st = ctx.enter_context(tc.tile_pool(name="ps_st", bufs=3, space="PSUM"))
        ps_av = ctx.enter_context(tc.tile_pool(name="ps_av", bufs=1, space="PSUM"))
        ps_mx = ctx.enter_context(tc.tile_pool(name="ps_mx", bufs=1, space="PSUM"))

        nc.vector.memset(e_f32, 0.0)
        nc.vector.memset(e_f32[HD : HD + 1, :], 1.0)
        nc.vector.tensor_copy(out=e_sb, in_=e_f32)
        # rows 65..127 of the broadcast rhs must be real zeros: the K=128
        # matmul streams all 128 partitions on hardware
        nc.vector.memset(z_f32, 0.0)
        for t in rcp_rs:
            nc.vector.tensor_copy(out=t, in_=z_f32)

        # --- priority startup on the Act ring: its sequencer is idle while
        # the sync ring runs the kernel preamble (~7us), and each dma_start
        # costs ~0.65us of sequencer dispatch, so these are one-per-tensor.
        wqk_ts = {0: qkp.tile([P, CC, 2, P], F32R, tag="wqk", bufs=4, name="wqk_t")}
        qk_ts = {}
        for cc in range(CC):
            for kk, j in ((0, 0), (1, CC)):
                nc.sync.dma_start(
                    out=wqk_ts[0][:, cc, kk, :],
                    in_=wqkT_d[cc * P : (cc + 1) * P, j * P : (j + 1) * P],
                )
            nc.sync.dma_start(
                out=xT_sb[:, cc, 0:512], in_=xT_d[cc * P : (cc + 1) * P, 0:512]
            )
        # bulk on the sync ring, in need order
        nc.sync.dma_start(out=polT_sb, in_=polT_d[:])
        for cc in range(CC):
            nc.sync.dma_start(
                out=xT_sb[:, cc, 512:1024],
                in_=xT_d[cc * P : (cc + 1) * P, 512:1024],
            )

        # HAM warm-up: the PE clock-gate defaults to 1.2GHz and only reaches
        # 2.4GHz after ~3.4us of sustained matmul activity.  Burn dummy
        # matmuls on the first-arriving weight tile while the startup burst
        # streams in, so real matmuls run at full clock.
        with nc.named_scope("warmup"):
            ps_w = ps_mx.tile([P, 512], F32, tag="mx", name="ps_warm")
            for _ in range(30):
                nc.tensor.matmul(
                    ps_w[:, 0:128],
                    lhsT=wqk_ts[0][:, 0, 0, :],
                    rhs=wqk_ts[0][:, 0, 0, :],
                    start=True,
                    stop=True,
                )

        def emit_qk_mms(jq, nhs=(0, 1)):
            """qk^T matmuls for pair jq: qk_t[:,0,:] = q chunk jq,
            qk_t[:,1,:] = k chunk jq+6 (embedding dim on partitions)."""
            wqk_t, qk_t = wqk_ts[jq], qk_ts[jq]
            with nc.named_scope("qk_mm"):
                for kk in range(2):
                    for nh in nhs:
                        ps = ps_mx.tile([P, 512], F32, tag="mx", name="ps_qk")
                        for cc in range(CC):
                            nc.tensor.matmul(
                                ps,
                                lhsT=wqk_t[:, cc, kk, :],
                                rhs=xT_sb[:, cc, nh * 512 : (nh + 1) * 512],
                                start=(cc == 0),
                                stop=(cc == CC - 1),
                            )
                        nc.vector.tensor_copy(
                            out=qk_t[:, kk, nh * 512 : (nh + 1) * 512], in_=ps
                        )

        qk_ts[0] = qkp.tile([P, 2, N], BF16, tag="qkT", bufs=CC, name="qk_t")
        emit_qk_mms(0, nhs=(0,))
        # pol columns of v_aug: DVE free-dim broadcast copies
        for nch in range(NC_):
            nc.vector.tensor_copy(
                out=v_aug[:, nch, :, HD : HD + 1],
                in_=polT_sb[:, nch : nch + 1]
                .unsqueeze(1)
                .broadcast_to((P, H, 1)),
            )
        emit_qk_mms(0, nhs=(1,))

        # ---- v natural layout, pol-scaled, into v_aug --------------------
        with tc.tile_pool(name="phv", bufs=1) as phv:
            wv_sb = phv.tile([P, CC, H * HD], F32R)
            for cc in range(CC):
                nc.sync.dma_start(
                    out=wv_sb[:, cc], in_=wvT_d[cc * P : (cc + 1) * P, :]
                )
            nc.sync.dma_start(out=dmask_sb, in_=dmask_d[:])
            nc.sync.dma_start(out=b_sb, in_=bias_d[:].partition_broadcast(P))
            with nc.named_scope("v_mm"):
                for nch in range(NC_):
                    for ev0, ev_sz, h0 in ((0, 512, 0), (512, 256, 8)):
                        nheads = ev_sz // HD
                        ps = ps_st.tile([P, 2, 512], F32, tag="st", name="ps_v")
                        psv = ps[:, 0, :ev_sz]
                        for cc in range(CC):
                            nc.tensor.matmul(
                                psv,
                                lhsT=xT_sb[:, cc, nch * P : (nch + 1) * P],
                                rhs=wv_sb[:, cc, ev0 : ev0 + ev_sz],
                                start=(cc == 0),
                                stop=(cc == CC - 1),
                            )
                        nc.vector.tensor_mul(
                            out=v_aug[:, nch, h0 : h0 + nheads, 0:HD],
                            in0=psv.rearrange("p (h d) -> p h d", d=HD),
                            in1=polT_sb[:, nch : nch + 1]
                            .unsqueeze(1)
                            .broadcast_to((P, nheads, HD)),
                        )

        # ------------------- attention, nh-major --------------------------
        with tc.tile_pool(name="attn", bufs=1) as attn:
            pending_norm = [None]  # deferred norm closure for the prev head

            def proj_chunk(nch):
                with nc.named_scope("proj_mm"):
                    y_t = attn.tile([P, C], F32, tag="y", bufs=2, name="y_t")
                    for oi, (o0, o_sz) in enumerate(((0, 512), (512, 256))):
                        pool, tagn = (ps_mx, "mx") if oi == 0 else (ps_av, "av")
                        ps = pool.tile([P, 512], F32, tag=tagn, name="ps_yt")
                        psy = ps[:, :o_sz]
                        for ec in range(CC):
                            nc.tensor.matmul(
                                psy,
                                lhsT=outT[:, ec, nch * P : (nch + 1) * P],
                                rhs=pw_bf[:, ec, o0 : o0 + o_sz],
                                start=(ec == 0),
                                stop=(ec == CC - 1),
                            )
                        nc.vector.tensor_add(
                            out=y_t[:, o0 : o0 + o_sz],
                            in0=psy,
                            in1=b_sb[:, o0 : o0 + o_sz],
                        )
                    nc.sync.dma_start(out=y_d[nch * P : (nch + 1) * P, :], in_=y_t)

            for nh in range(NH):
                nsl = slice(nh * 512, (nh + 1) * 512)
                for h in range(H):
                    j = h // 2
                    hp = 64 * (h % 2)
                    if nh == 0 and h % 2 == 0 and j + 1 < CC:
                        # prefetch next q/k pair (DMA + matmuls) during pass 0
                        jn = j + 1
                        wqk_ts[jn] = qkp.tile(
                            [P, CC, 2, P], F32R, tag="wqk", bufs=4, name="wqk_t"
                        )
                        for kk, jj in ((0, jn), (1, jn + CC)):
                            for cc in range(CC):
                                nc.sync.dma_start(
                                    out=wqk_ts[jn][:, cc, kk, :],
                                    in_=wqkT_d[
                                        cc * P : (cc + 1) * P,
                                        jj * P : (jj + 1) * P,
                                    ],
                                )
                        qk_ts[jn] = qkp.tile(
                            [P, 2, N], BF16, tag="qkT", bufs=CC, name="qk_t"
                        )
                        emit_qk_mms(jn)
                    qk_pair = qk_ts[j]
                    av = ps_av.tile([HD + 1, 512], F32, tag="av", name="av")
                    E_ts = []
                    # chunk-pair pipeline: st(t) || exp(t-1..2) || av(t-3)
                    for t in range(NC_ // 2 + 3):
                        if t == 3 and pending_norm[0] is not None:
                            # prev head's norm, emitted once this head's st
                            # matmuls are queued so the PE never waits on it
                            pending_norm[0]()
                            pending_norm[0] = None
                        if t < NC_ // 2:
                            st = ps_st.tile([P, 2, 512], F32, tag="st", name="st")
                            E_t = attn.tile(
                                [P, 2, 512], BF16, tag="E", bufs=5, name="E_t"
                            )
                            with nc.named_scope("st_mm"):
                                for k in range(2):
                                    mc = 2 * t + k
                                    nc.tensor.matmul(
                                        st[:, k, :],
                                        lhsT=qk_pair[
                                            hp : hp + HD, 1, mc * P : (mc + 1) * P
                                        ],
                                        rhs=qk_pair[hp : hp + HD, 0, nsl],
                                        start=True,
                                        stop=True,
                                    )
                            nc.scalar.activation(
                                out=E_t,
                                in_=st,
                                func=mybir.ActivationFunctionType.Exp,
                                scale=SCALE,
                            )
                            if t in (2 * nh, 2 * nh + 1):
                                # chunks 2t, 2t+1 hold the diagonal
                                diag_ap = bass.AP(
                                    tensor=E_t.tensor,
                                    offset=E_t.offset + (2 * t * P - 512 * nh),
                                    ap=[E_t.ap[0], [512 + P, 2], [1, P]],
                                )
                                nc.vector.tensor_mul(
                                    out=diag_ap,
                                    in0=diag_ap,
                                    in1=dmask_sb[:, 2 * t : 2 * t + 2, :],
                                )
                            E_ts.append(E_t)
                        if t >= 3:
                            ta = t - 3
                            E_a = E_ts[ta]
                            with nc.named_scope("av_mm"):
                                for k in range(2):
                                    mc = 2 * ta + k
                                    nc.tensor.matmul(
                                        av,
                                        lhsT=v_aug[:, mc, h, 0 : HD + 1],
                                        rhs=E_a[:, k, :],
                                        start=(mc == 0),
                                        stop=(mc == NC_ - 1),
                                    )
                    av_sb = attn.tile(
                        [HD + 1, 512], F32, tag="avsb", bufs=3, name="av_sb"
                    )
                    nc.vector.tensor_copy(out=av_sb, in_=av)

                    def make_norm(av_sb=av_sb, h=h, nh=nh, nsl=nsl):
                        def norm():
                            with nc.named_scope("norm"):
                                # rcp rows 0..63 are finite filler (the av
                                # values) so the K=65 broadcast matmul below
                                # never multiplies 0 by uninitialized bits
                                rcp = attn.tile(
                                    [HD + 1, 512], F32, tag="rcp", bufs=2,
                                    name="rcp",
                                )
                                # full-tile op from partition 0: the custom
                                # DVE op returns zeros on HW when started at
                                # a partition offset.  Rows 0..63 are 1/av --
                                # finite filler, zeroed by the selector.
                                nc.vector.reciprocal_approx_fast(
                                    out=rcp, in_=av_sb
                                )
                                # round to f32r on the (lightly loaded) Act
                                # engine; the f32r matmul requires it
                                rcp_r = rcp_rs[h % 2]
                                nc.scalar.activation(
                                    out=rcp_r[0 : HD + 1, :],
                                    in_=rcp,
                                    func=mybir.ActivationFunctionType.Copy,
                                )
                                r_ps = ps_mx.tile(
                                    [P, 512], F32, tag="mx", name="r_ps"
                                )
                                nc.tensor.matmul(
                                    r_ps[0:HD, :],
                                    lhsT=e_sb,
                                    rhs=rcp_r,
                                    start=True,
                                    stop=True,
                                )
                                qj = h // 2
                                if h % 2 == 0:
                                    nc.vector.tensor_mul(
                                        out=outT[0:HD, qj, nsl],
                                        in0=av_sb[0:HD, :],
                                        in1=r_ps[0:HD, :],
                                    )
                                else:
                                    tmp = attn.tile(
                                        [HD, 512], BF16, tag="otmp", bufs=2,
                                        name="tmp",
                                    )
                                    nc.vector.tensor_mul(
                                        out=tmp,
                                        in0=av_sb[0:HD, :],
                                        in1=r_ps[0:HD, :],
                                    )
                                    nc.sync.dma_start(
                                        out=outT[HD:P, qj, nsl], in_=tmp
                                    )

                        return norm

                    pending_norm[0] = make_norm()
                    if nh == 0 and h == 0:
                        # projection weights, then a one-time bf16 cast so
                        # proj matmuls take half-width LDWEIGHTS
                        for cc in range(CC):
                            nc.sync.dma_start(
                                out=pw_sb[:, cc],
                                in_=pwT_d[cc * P : (cc + 1) * P, :],
                            )
                            nc.vector.tensor_copy(
                                out=pw_bf[:, cc],
                                in_=pw_sb[:, cc].bitcast(F32),
                            )
                    if nh == 1 and h % 3 == 1:
                        # interleave pass-0 projection rows into pass 1
                        proj_chunk(h // 3)
                # (pending norm for h==11 is flushed inside the next pass /
                # tail below)
            # keep the PE clock warm through the final norm latency chain
            with nc.named_scope("warmkeep"):
                ps_w2 = ps_mx.tile([P, 512], F32, tag="mx", name="ps_wk")
                for _ in range(6):
                    nc.tensor.matmul(
                        ps_w2[0:HD, :],
                        lhsT=e_sb,
                        rhs=rcp_rs[0],
                        start=True,
                        stop=True,
                    )
            pending_norm[0]()
            pending_norm[0] = None
            for nch in range(4, NC_):
                proj_chunk(nch)

    nc.compile()
    return nc


_NC_CACHE = None


def _get_nc():
    global _NC_CACHE
    if _NC_CACHE is None:
        _NC_CACHE = _build_nc()
    return _NC_CACHE


def kernel(x, policy, qkv_w, proj_w, proj_b):
    global LAST_RESULTS
    x = np.asarray(x, dtype=np.float32)
    policy = np.asarray(policy, dtype=np.float32)
    qkv_w = np.asarray(qkv_w, dtype=np.float32)
    proj_w = np.asarray(proj_w, dtype=np.float32)
    proj_b = np.asarray(proj_b, dtype=np.float32)

    wqkT = np.ascontiguousarray(qkv_w[: 2 * H * HD].T)  # [768, 1536]
    wvT = np.ascontiguousarray(qkv_w[2 * H * HD :].T)  # [768, 768]
    pwT = np.ascontiguousarray(proj_w.T)  # [768, 768]

    in_maps = []
    for b in range(B):
        pol = policy[b, :, 0]
        polc = np.maximum(pol, 1e-30)
        # [p, chunk] layout: global n = chunk*128 + p
        polT = np.ascontiguousarray(pol.reshape(NC_, P).T)
        dmask = np.ones((P, NC_, P), dtype=np.float32)
        rng = np.arange(P)
        for kch in range(NC_):
            dmask[rng, kch, rng] = 1.0 / polc[kch * P + rng]
        in_maps.append(
            dict(
                xT=np.ascontiguousarray(x[b].T),
                wqkT=wqkT,
                wvT=wvT,
                pwT=pwT,
                bias=proj_b,
                polT=polT.astype(np.float32),
                dmask=dmask,
            )
        )

    nc = _get_nc()
    trace = os.environ.get("KERNEL_TRACE", "0") == "1"
    res = run_bass_kernel_spmd(
        nc,
        in_maps,
        core_ids=list(range(B)),
        trace=trace,
        trace_cores=list(range(B)) if trace else None,
        stitch_traces=False,
    )
    LAST_RESULTS = res
    return np.stack([res.results[b]["y"] for b in range(B)], axis=0)
